# revision 30
# baseline (speedup 1.0000x reference)
"""Trainium2 Bass kernel for the nn_MultiHeadAttention problem.

Data-parallel over batch: each of the 8 NeuronCores processes one batch
element independently (no collectives).

Mask compaction: the host gathers only the valid query/key positions
(QMask/KMask true), padded to a multiple of 128, and scatters the
output back (masked query rows are exactly zero in the reference).
With ~50% random masks this cuts the attention work ~4x.  If the max
query count only slightly exceeds a 512 multiple, the device is capped
there and the few overflow queries are computed exactly on the host.

V2 engine balance (per core, Lq=512, Lk=640, H=16 heads):
  PE     ~54us: proj + scores + PV + out-proj, software-pipelined so it
         streams continuously (HAM stays warm at 2.4 GHz).
  ACT    ~48us: exp ONLY, merged into 3 ACTIVATEs per head over 2-bank
         [128,1024] PSUM reads to amortize the ~293ns/instr overhead.
  DVE    ~50us: all PSUM evacuation (proj/vproj/pv/den), batched recip,
         batch-1/2 normalize muls, ysum copies/adds.
  GPSIMD: v2 ones-column copies, batch-0 normalize muls, some DMA
         triggers (obs, rbounce/bcs b0, den h<8).
  Sync:  input chunk DMA triggers, den h>=8, Y out.

Normalize: denominator rows DMA-transposed into dstacks, batched
reciprocal on DVE; recip broadcast across partitions via one batched
DRAM-bounce DMA per batch (b0, b1); the final batch (last 2 heads) uses
tiny PE one-hot matmuls into PSUM instead, killing the DMA round-trip
latency on the tail.  Y is written bf16 per q-tile (host casts to f32).
"""

import math
import os
import sys

import numpy as np

try:
    import concourse  # noqa: F401
except ImportError:  # pragma: no cover
    for _p in ("/opt/trn_rl_repo", os.path.expanduser("~/.axon_site/_ro/trn_rl_repo")):
        if os.path.isdir(_p) and _p not in sys.path:
            sys.path.insert(0, _p)

import ml_dtypes

import concourse.bass as bass
import concourse.tile as tile
from concourse import bacc, mybir

B, L, E, H, D = 8, 1024, 1024, 16, 64
P = 128          # partitions
NCH = E // P     # 8 e-chunks (2 heads each)
F32 = mybir.dt.float32
BF16 = mybir.dt.bfloat16


def _chunks(n, step=512):
    return [(s, min(s + step, n)) for s in range(0, n, step)]


def build_bass(ntq, ntk):
    Lq, Lk = ntq * P, ntk * P
    nc = bacc.Bacc(None, target_bir_lowering=False, debug=False)

    Lqkv = Lq + 2 * Lk
    QKV = nc.declare_dram_parameter("QKV", [E, Lqkv], BF16, isOutput=False)
    W2 = nc.declare_dram_parameter("W2", [P, NCH, P], BF16, isOutput=False)
    OB = nc.declare_dram_parameter("OB", [E, E], BF16, isOutput=False)
    KM = nc.declare_dram_parameter("KM", [P, ntk], F32, isOutput=False)
    Y = nc.declare_dram_parameter("Y", [Lq, E], BF16, isOutput=True)
    CT7 = nc.declare_dram_parameter("CT7", [P, Lq], BF16, isOutput=True)
    DEN = nc.declare_dram_parameter("DEN", [2, Lq], F32, isOutput=True)
    rbounce = nc.dram_tensor("rbounce", [H, Lq], BF16)

    # normalize batches: heads [h0,h1) of batches 0/1 are normalized on
    # device once their PVs land; the final 2 heads stay UNNORMALIZED on
    # device (ct7/dens shipped out, the host applies the exact rank-128
    # correction to Y), killing the tail's normalize latency chain.
    NB = [(0, H // 2, None), (H // 2, H - 2, None), (H - 2, H, None)]

    with tile.TileContext(nc) as tc:
        with (
            tc.tile_pool(name="singles", bufs=1) as singles,
            tc.tile_pool(name="qkT", bufs=2) as qkT,
            tc.tile_pool(name="vaug", bufs=2) as vaug,
            tc.tile_pool(name="ppool", bufs=2) as ppool,
            tc.tile_pool(name="ystage", bufs=2) as ystage,
            tc.tile_pool(name="dtpool", bufs=2) as dtpool,
            tc.tile_pool(name="psbig", bufs=2, space="PSUM") as psbig,
            tc.tile_pool(name="pspv", bufs=2, space="PSUM") as pspv,
            tc.tile_pool(name="pssmall", bufs=2, space="PSUM") as pssmall,
        ):
            # --- persistent SBUF tensors -------------------------------
            qkvts = singles.tile([P, NCH, Lqkv], BF16)
            qts = qkvts[:, :, 0:Lq]
            kts = qkvts[:, :, Lq:Lq + Lk]
            vts = qkvts[:, :, Lq + Lk:Lqkv]
            obs = singles.tile([P, NCH, E], BF16)
            w2s = singles.tile([P, NCH, P], BF16)
            kms = singles.tile([P, ntk], F32)
            ct = singles.tile([P, NCH, Lq], BF16)
            dstacks, rstacks, bstacks = [], [], []
            for bi, (h0, h1, _) in enumerate(NB[:2]):
                nh = h1 - h0
                ds = singles.tile([nh * ntq, P], F32, tag=f"ds{bi}")
                rs = singles.tile([nh * ntq, P], BF16, tag=f"rs{bi}")
                dstacks.append(ds)
                rstacks.append(rs)
                bs = singles.tile([P, nh, Lq], BF16, tag=f"bs{bi}")
                bstacks.append(bs)

            # --- table load + input DMAs -------------------------------
            # dummy exp first so the ~2.7us ACT table load overlaps DMAs
            dume = singles.tile([P, 8], BF16)
            nc.vector.memset(dume[:], 0.0)
            nc.scalar.activation(out=dume[:], in_=dume[:],
                                 func=mybir.ActivationFunctionType.Exp)
            nc.sync.dma_start(out=w2s[:], in_=W2[:])
            nc.sync.dma_start(out=kms[:], in_=KM[:])
            # PE warmup: cold dummy matmuls so the HAM clock gate opens
            # while the first chunk DMAs land (~3us of cold N=256 MMs)
            warm = singles.tile([P, 256], BF16)
            nc.vector.memset(warm[:], 0.0)
            for wi in range(14):
                wps = pssmall.tile([P, 512], F32, tag="small")
                nc.tensor.matmul(out=wps[:, 0:256], lhsT=warm[:, 0:128],
                                 rhs=warm[:], start=True, stop=True)
            for c in range(NCH):
                nc.sync.dma_start(out=qkvts[:, c, :],
                                  in_=QKV[c * P:(c + 1) * P, :])

            def batch_of(h):
                return next(i for i, (a, b2, _) in enumerate(NB) if a <= h < b2)

            def emit_pv(h):
                """PV for head h (pt/v2 already computed), plus evacuation."""
                c, hf = h // 2, h % 2
                pv = pspv.tile([65, Lq], F32, tag="pv")
                for kt in range(ntk):
                    nc.tensor.matmul(
                        out=pv[:],
                        lhsT=v2s[c % 2][:, kt, 65 * hf:65 * hf + 65],
                        rhs=pts[c % 2][hf][:, kt, :],
                        start=(kt == 0), stop=(kt == ntk - 1),
                    )
                # evacuate: unnormalized C^T rows + denominator row
                nc.vector.tensor_copy(ct[64 * hf:64 * hf + 64, c, :], pv[0:64, :])
                dtmp = dtpool.tile([1, Lq], F32)
                nc.vector.tensor_copy(dtmp[:], pv[64:65, :])
                bi = batch_of(h)
                hrel = h - NB[bi][0]
                if bi < 2:
                    eng = nc.gpsimd if h < H // 2 else nc.sync
                    eng.dma_start(
                        out=dstacks[bi][hrel * ntq:(hrel + 1) * ntq, :],
                        in_=dtmp[:])
                else:
                    # last 2 heads: denominator goes to the host
                    nc.sync.dma_start(out=DEN[hrel:hrel + 1, :], in_=dtmp[:])

            def emit_norm_batch(bi):
                h0, h1, _ = NB[bi]
                nh = h1 - h0
                with nc.allow_low_precision(reason="softmax recip bf16"):
                    nc.vector.reciprocal(out=rstacks[bi][:], in_=dstacks[bi][:])
                # DRAM bounce + ONE batched broadcast DMA for the batch
                eng = nc.gpsimd if bi == 0 else nc.sync
                eng.dma_start(out=rbounce[h0:h1, :], in_=rstacks[bi][:])
                src = rbounce[h0:h1, :]
                bc_in = bass.AP(
                    tensor=src.tensor, offset=src.offset,
                    ap=[[0, P], [Lq, nh], [1, Lq]])
                eng.dma_start(out=bstacks[bi][:], in_=bc_in)
                for h in range(h0, h1):
                    c, hf = h // 2, h % 2
                    sl = ct[64 * hf:64 * hf + 64, c, :]
                    bsl = bstacks[bi][64 * hf:64 * hf + 64, h - h0, :]
                    # b0 muls ride the idle gpsimd engine; b1's gate partA
                    # so split them DVE/gpsimd to halve the chain's wall
                    meng = (nc.gpsimd if bi == 0 or (h - h0) % 2 == 0
                            else nc.vector)
                    meng.tensor_mul(sl, sl, bsl)

            # --- main loop over e-chunks (2 heads each) ----------------
            v2s = [None, None]   # v2 tiles by chunk parity
            pts = [[None, None], [None, None]]  # pt tiles [c%2][hf]
            prev_h = None
            for c in range(NCH):
                # output-proj weights trickle in behind the critical inputs
                nc.gpsimd.dma_start(out=obs[:, c, :], in_=OB[c * P:(c + 1) * P, :])
                # fused q/k projection for both heads of this chunk
                qkt2 = qkT.tile([P, Lq + Lk], BF16, tag="qkt2")
                for s0, s1 in _chunks(Lq + Lk):
                    ps = pssmall.tile([P, 512], F32, tag="small")
                    if s1 <= Lq:
                        nc.tensor.matmul(
                            out=ps[:, 0:s1 - s0], lhsT=w2s[:, c, :],
                            rhs=qts[:, c, s0:s1], start=True, stop=True)
                    elif s0 >= Lq:
                        nc.tensor.matmul(
                            out=ps[:, 0:s1 - s0], lhsT=w2s[:, c, :],
                            rhs=kts[:, c, s0 - Lq:s1 - Lq], start=True, stop=True)
                    else:
                        mid = Lq - s0
                        nc.tensor.matmul(
                            out=ps[:, 0:mid], lhsT=w2s[:, c, :],
                            rhs=qts[:, c, s0:Lq], start=True, stop=True)
                        nc.tensor.matmul(
                            out=ps[:, mid:s1 - s0], lhsT=w2s[:, c, :],
                            rhs=kts[:, c, 0:s1 - Lq], start=True, stop=True)
                    nc.vector.tensor_copy(qkt2[:, s0:s1], ps[:, 0:s1 - s0])
                qt2 = qkt2[:, 0:Lq]
                kt2 = qkt2[:, Lq:Lq + Lk]

                # v projection: k-tiles in 512-wide PSUM groups, evacuated
                # with strided APs into the [65,2] interleaved v2 layout
                v2 = vaug.tile([P, ntk, 130], BF16)
                v2s[c % 2] = v2
                for t0, t1 in _chunks(ntk * P):
                    ps = pssmall.tile([P, 512], F32, tag="small")
                    for t in range(t0 // P, (t1 + P - 1) // P):
                        nc.tensor.matmul(
                            out=ps[:, t * P - t0:(t + 1) * P - t0],
                            lhsT=vts[:, c, t * P:(t + 1) * P],
                            rhs=w2s[:, c, :],
                            start=True, stop=True,
                        )
                    nt = (t1 - t0) // P
                    base = v2[:, t0 // P, 0:64]
                    vt_out = bass.AP(
                        tensor=base.tensor, offset=base.offset,
                        ap=[list(base.ap[0]), [130, nt], [65, 2], [1, 64]])
                    nc.vector.tensor_copy(
                        vt_out,
                        ps[:, 0:t1 - t0].rearrange(
                            "p (nt two d) -> p nt two d", nt=nt, two=2))
                # denominator "ones" columns = slot-validity mask
                nc.gpsimd.tensor_copy(v2[:, :, 64], kms[:, :])
                nc.gpsimd.tensor_copy(v2[:, :, 129], kms[:, :])

                for hf in range(2):
                    h = 2 * c + hf
                    hq = qt2[64 * hf:64 * hf + 64, :]
                    hk = kt2[64 * hf:64 * hf + 64, :]
                    # scores (transposed, [k, q]) in 2-bank tiles + merged exp
                    pt = ppool.tile([P, ntk, Lq], BF16, tag=f"pt{hf}")
                    pts[c % 2][hf] = pt
                    for t0 in range(0, ntk, 2):
                        t1 = min(t0 + 2, ntk)
                        sps = psbig.tile([P, 1024], F32, tag="big")
                        for t in range(t0, t1):
                            for s0, s1 in _chunks(Lq):
                                nc.tensor.matmul(
                                    out=sps[:, (t - t0) * Lq + s0:(t - t0) * Lq + s1],
                                    lhsT=hk[:, t * P:(t + 1) * P],
                                    rhs=hq[:, s0:s1],
                                    start=True, stop=True,
                                )
                        w = (t1 - t0) * Lq
                        nc.scalar.activation(
                            out=pt[:, t0:t1, :], in_=sps[:, 0:w],
                            func=mybir.ActivationFunctionType.Exp,
                            scale=0.125,
                        )
                    # software pipeline: PV runs one head behind scores
                    if prev_h is not None:
                        emit_pv(prev_h)
                        for bi, (_, b1_, _) in enumerate(NB[:2]):
                            if prev_h == b1_ - 1:
                                emit_norm_batch(bi)
                    prev_h = h

            emit_pv(prev_h)
            # ship the unnormalized last chunk to the host for correction
            nc.sync.dma_start(out=CT7[:], in_=ct[:, NCH - 1, :])

            # output projection over chunks 0..NCH-2 (all device-normalized;
            # the host adds the last chunk's term from CT7/DEN).  Per-MM
            # subtile deps let early chunks' matmuls fill PE gaps during
            # the last heads' ACT-paced attention.
            for t in range(ntq):
                ya = psbig.tile([P, 1024], F32, tag="big")
                for c in range(NCH - 1):
                    for eh in range(2):
                        nc.tensor.matmul(
                            out=ya[:, 512 * eh:512 * (eh + 1)],
                            lhsT=ct[:, c, t * P:(t + 1) * P],
                            rhs=obs[:, c, 512 * eh:512 * (eh + 1)],
                            start=(c == 0), stop=(c == NCH - 2),
                        )
                ys = ystage.tile([P, E], BF16, tag="ys")
                if t % 2 == 0:
                    nc.scalar.copy(ys[:], ya[:])
                else:
                    nc.vector.tensor_copy(ys[:], ya[:])
                nc.sync.dma_start(out=Y[t * P:(t + 1) * P, :], in_=ys[:])

    nc.compile()
    return nc


def make_core_inputs(Q, K, V, HeadLinear, OutputLiner, QMask, KMask):
    """Host-side sharding/compaction (see module docstring)."""
    bf16 = ml_dtypes.bfloat16
    qm = np.asarray(QMask).astype(bool)
    km = np.asarray(KMask).astype(bool)
    qidxs = [np.nonzero(qm[b])[0] for b in range(B)]
    kidxs = [np.nonzero(km[b])[0] for b in range(B)]
    maxq = max(len(ix) for ix in qidxs)
    qcap = maxq
    if maxq > 512 and maxq % 512 <= 64:
        qcap = (maxq // 512) * 512
    qidxs = [ix[:qcap] for ix in qidxs]
    ntq = max(1, math.ceil(max(len(ix) for ix in qidxs) / P))
    ntk = max(1, math.ceil(max(len(ix) for ix in kidxs) / P))
    Lq, Lk = ntq * P, ntk * P

    w2 = np.zeros((P, NCH, P), dtype=np.float32)
    hl = np.asarray(HeadLinear, dtype=np.float32)
    for c in range(NCH):
        w2[0:64, c, 0:64] = hl[2 * c]
        w2[64:128, c, 64:128] = hl[2 * c + 1]
    w2b = w2.astype(bf16)
    ob = np.asarray(OutputLiner, dtype=np.float32).astype(bf16)

    in_maps = []
    for b in range(B):
        qi, ki = qidxs[b], kidxs[b]
        qkv = np.zeros((Lq + 2 * Lk, E), dtype=np.float32)
        qkv[:len(qi)] = np.asarray(Q[b], dtype=np.float32)[qi]
        qkv[Lq:Lq + len(ki)] = np.asarray(K[b], dtype=np.float32)[ki]
        qkv[Lq + Lk:Lq + Lk + len(ki)] = np.asarray(V[b], dtype=np.float32)[ki]
        kmc = np.zeros(Lk, dtype=np.float32)
        kmc[:len(ki)] = 1.0
        in_maps.append({
            "QKV": np.ascontiguousarray(qkv.T.astype(bf16)),
            "W2": w2b, "OB": ob,
            "KM": np.ascontiguousarray(kmc.reshape(ntk, P).T),
        })
    return in_maps, qidxs, ntq, ntk


_NC_CACHE = {}


def _get_nc(ntq, ntk):
    if (ntq, ntk) not in _NC_CACHE:
        _NC_CACHE[(ntq, ntk)] = build_bass(ntq, ntk)
    return _NC_CACHE[(ntq, ntk)]


def _host_tail(Q, K, V, HeadLinear, OutputLiner, KMask, b, tidx):
    """Exact fp32 attention for a few overflow queries of batch b."""
    hl = np.asarray(HeadLinear, dtype=np.float32)
    ob = np.asarray(OutputLiner, dtype=np.float32)
    ki = np.nonzero(np.asarray(KMask[b]).astype(bool))[0]
    q = np.asarray(Q[b], dtype=np.float32)[tidx]
    kk = np.asarray(K[b], dtype=np.float32)[ki]
    vv = np.asarray(V[b], dtype=np.float32)[ki]
    outs = []
    for h in range(H):
        sl = slice(h * D, (h + 1) * D)
        qh = q[:, sl] @ hl[h]
        kh = kk[:, sl] @ hl[h]
        vh = vv[:, sl] @ hl[h]
        s = (qh @ kh.T) / np.float32(np.sqrt(D))
        s -= s.max(axis=1, keepdims=True)
        p = np.exp(s)
        p /= p.sum(axis=1, keepdims=True)
        outs.append(p @ vh)
    return np.concatenate(outs, axis=1) @ ob


def kernel(Q, K, V, HeadLinear, OutputLiner, QMask, KMask):
    from concourse.bass_utils import run_bass_kernel_spmd

    in_maps, qidxs, ntq, ntk = make_core_inputs(
        Q, K, V, HeadLinear, OutputLiner, QMask, KMask)
    nc = _get_nc(ntq, ntk)
    res = run_bass_kernel_spmd(nc, in_maps, list(range(B)))
    out = np.zeros((B, L, E), dtype=np.float32)
    qm = np.asarray(QMask).astype(bool)
    bf16 = ml_dtypes.bfloat16
    # last-chunk normalize correction: the device used UNNORMALIZED ct
    # for the final e-chunk's contribution; add (ctN - ctU)^T @ OB_rows
    ob7 = np.asarray(OutputLiner, dtype=np.float32).astype(bf16).astype(
        np.float32)[(NCH - 1) * P:, :]
    for b in range(B):
        yc = np.asarray(res.results[b]["Y"]).astype(np.float32)
        ct7 = np.asarray(res.results[b]["CT7"]).astype(np.float32)
        den = np.asarray(res.results[b]["DEN"]).astype(np.float32)
        scale = np.repeat(1.0 / den, 64, axis=0)            # [128, Lq]
        yc = yc + (ct7 * scale).T @ ob7
        out[b][qidxs[b]] = yc[:len(qidxs[b])]
        full = np.nonzero(qm[b])[0]
        tidx = full[len(qidxs[b]):]
        if len(tidx):
            out[b][tidx] = _host_tail(
                Q, K, V, HeadLinear, OutputLiner, KMask, b, tidx)
    return out


# revision 37
# speedup vs baseline: 1.0059x; 1.0059x over previous
"""Trainium2 Bass kernel for the nn_MultiHeadAttention problem.

Data-parallel over batch: each of the 8 NeuronCores processes one batch
element independently (no collectives).

Mask compaction: the host gathers only the valid query/key positions
(QMask/KMask true), padded to a multiple of 128, and scatters the
output back (masked query rows are exactly zero in the reference).
With ~50% random masks this cuts the attention work ~4x.  If the max
query count only slightly exceeds a 512 multiple, the device is capped
there and the few overflow queries are computed exactly on the host.

V2 engine balance (per core, Lq=512, Lk=640, H=16 heads):
  PE     ~54us: proj + scores + PV + out-proj, software-pipelined so it
         streams continuously (HAM stays warm at 2.4 GHz).
  ACT    ~48us: exp ONLY, merged into 3 ACTIVATEs per head over 2-bank
         [128,1024] PSUM reads to amortize the ~293ns/instr overhead.
  DVE    ~50us: all PSUM evacuation (proj/vproj/pv/den), batched recip,
         batch-1/2 normalize muls, ysum copies/adds.
  GPSIMD: v2 ones-column copies, batch-0 normalize muls, some DMA
         triggers (obs, rbounce/bcs b0, den h<8).
  Sync:  input chunk DMA triggers, den h>=8, Y out.

Normalize: denominator rows DMA-transposed into dstacks, batched
reciprocal on DVE; recip broadcast across partitions via one batched
DRAM-bounce DMA per batch (b0, b1); the final batch (last 2 heads) uses
tiny PE one-hot matmuls into PSUM instead, killing the DMA round-trip
latency on the tail.  Y is written bf16 per q-tile (host casts to f32).
"""

import math
import os
import sys

import numpy as np

try:
    import concourse  # noqa: F401
except ImportError:  # pragma: no cover
    for _p in ("/opt/trn_rl_repo", os.path.expanduser("~/.axon_site/_ro/trn_rl_repo")):
        if os.path.isdir(_p) and _p not in sys.path:
            sys.path.insert(0, _p)

import ml_dtypes

import concourse.bass as bass
import concourse.tile as tile
from concourse import bacc, mybir

B, L, E, H, D = 8, 1024, 1024, 16, 64
P = 128          # partitions
NCH = E // P     # 8 e-chunks (2 heads each)
F32 = mybir.dt.float32
BF16 = mybir.dt.bfloat16


def _chunks(n, step=512):
    return [(s, min(s + step, n)) for s in range(0, n, step)]


def build_bass(ntq, ntk):
    Lq, Lk = ntq * P, ntk * P
    nc = bacc.Bacc(None, target_bir_lowering=False, debug=False)

    Lqkv = Lq + 2 * Lk
    QKV = nc.declare_dram_parameter("QKV", [E, Lqkv], BF16, isOutput=False)
    W2 = nc.declare_dram_parameter("W2", [P, NCH, P], BF16, isOutput=False)
    OB = nc.declare_dram_parameter("OB", [E, E], BF16, isOutput=False)
    KM = nc.declare_dram_parameter("KM", [P, ntk], F32, isOutput=False)
    Y = nc.declare_dram_parameter("Y", [Lq, E], BF16, isOutput=True)
    CT7 = nc.declare_dram_parameter("CT7", [P, Lq], BF16, isOutput=True)
    DEN = nc.declare_dram_parameter("DEN", [2, Lq], F32, isOutput=True)
    rbounce = nc.dram_tensor("rbounce", [H, Lq], BF16)

    # normalize batches: heads [h0,h1) of batches 0/1 are normalized on
    # device once their PVs land; the final 2 heads stay UNNORMALIZED on
    # device (ct7/dens shipped out, the host applies the exact rank-128
    # correction to Y), killing the tail's normalize latency chain.
    NB = [(0, H // 2, None), (H // 2, H - 2, None), (H - 2, H, None)]

    with tile.TileContext(nc) as tc:
        with (
            tc.tile_pool(name="singles", bufs=1) as singles,
            tc.tile_pool(name="qkT", bufs=2) as qkT,
            tc.tile_pool(name="vaug", bufs=2) as vaug,
            tc.tile_pool(name="ppool", bufs=2) as ppool,
            tc.tile_pool(name="ystage", bufs=4) as ystage,
            tc.tile_pool(name="dtpool", bufs=2) as dtpool,
            tc.tile_pool(name="psbig", bufs=2, space="PSUM") as psbig,
            tc.tile_pool(name="pspv", bufs=2, space="PSUM") as pspv,
            tc.tile_pool(name="pssmall", bufs=2, space="PSUM") as pssmall,
        ):
            # --- persistent SBUF tensors -------------------------------
            qkvts = singles.tile([P, NCH, Lqkv], BF16)
            qts = qkvts[:, :, 0:Lq]
            kts = qkvts[:, :, Lq:Lq + Lk]
            vts = qkvts[:, :, Lq + Lk:Lqkv]
            obs = singles.tile([P, NCH, E], BF16)
            w2s = singles.tile([P, NCH, P], BF16)
            kms = singles.tile([P, ntk], F32)
            ct = singles.tile([P, NCH, Lq], BF16)
            dstacks, rstacks, bstacks = [], [], []
            for bi, (h0, h1, _) in enumerate(NB[:2]):
                nh = h1 - h0
                ds = singles.tile([nh * ntq, P], F32, tag=f"ds{bi}")
                rs = singles.tile([nh * ntq, P], BF16, tag=f"rs{bi}")
                dstacks.append(ds)
                rstacks.append(rs)
                bs = singles.tile([P, nh, Lq], BF16, tag=f"bs{bi}")
                bstacks.append(bs)

            # --- table load + input DMAs -------------------------------
            # dummy exp first so the ~2.7us ACT table load overlaps DMAs
            dume = singles.tile([P, 8], BF16)
            nc.vector.memset(dume[:], 0.0)
            nc.scalar.activation(out=dume[:], in_=dume[:],
                                 func=mybir.ActivationFunctionType.Exp)
            nc.sync.dma_start(out=w2s[:], in_=W2[:])
            nc.sync.dma_start(out=kms[:], in_=KM[:])
            # PE warmup: cold dummy matmuls so the HAM clock gate opens
            # while the first chunk DMAs land (~3us of cold N=256 MMs)
            warm = singles.tile([P, 256], BF16)
            nc.vector.memset(warm[:], 0.0)
            for wi in range(14):
                wps = pssmall.tile([P, 512], F32, tag="small")
                nc.tensor.matmul(out=wps[:, 0:256], lhsT=warm[:, 0:128],
                                 rhs=warm[:], start=True, stop=True)
            # inputs striped over 4 DMA queues — a single queue moves only
            # ~130 GB/s and would pace (and HAM-chill) the whole first half
            qengs = [nc.sync, nc.gpsimd, nc.scalar]
            for c in range(NCH):
                qengs[c % 3].dma_start(out=qkvts[:, c, :],
                                       in_=QKV[c * P:(c + 1) * P, :])

            def batch_of(h):
                return next(i for i, (a, b2, _) in enumerate(NB) if a <= h < b2)

            def emit_pv(h):
                """PV for head h (pt/v2 already computed), plus evacuation."""
                c, hf = h // 2, h % 2
                pv = pspv.tile([65, Lq], F32, tag="pv")
                for kt in range(ntk):
                    nc.tensor.matmul(
                        out=pv[:],
                        lhsT=v2s[c % 2][:, kt, 65 * hf:65 * hf + 65],
                        rhs=pts[c % 2][hf][:, kt, :],
                        start=(kt == 0), stop=(kt == ntk - 1),
                    )
                # evacuate: unnormalized C^T rows + denominator row
                nc.vector.tensor_copy(ct[64 * hf:64 * hf + 64, c, :], pv[0:64, :])
                dtmp = dtpool.tile([1, Lq], F32)
                nc.vector.tensor_copy(dtmp[:], pv[64:65, :])
                bi = batch_of(h)
                hrel = h - NB[bi][0]
                if bi < 2:
                    eng = nc.gpsimd if h < H // 2 else nc.sync
                    eng.dma_start(
                        out=dstacks[bi][hrel * ntq:(hrel + 1) * ntq, :],
                        in_=dtmp[:])
                else:
                    # last 2 heads: denominator goes to the host
                    nc.sync.dma_start(out=DEN[hrel:hrel + 1, :], in_=dtmp[:])

            def emit_norm_batch(bi):
                h0, h1, _ = NB[bi]
                nh = h1 - h0
                with nc.allow_low_precision(reason="softmax recip bf16"):
                    nc.vector.reciprocal(out=rstacks[bi][:], in_=dstacks[bi][:])
                # DRAM bounce + ONE batched broadcast DMA for the batch
                eng = nc.gpsimd if bi == 0 else nc.sync
                eng.dma_start(out=rbounce[h0:h1, :], in_=rstacks[bi][:])
                src = rbounce[h0:h1, :]
                bc_in = bass.AP(
                    tensor=src.tensor, offset=src.offset,
                    ap=[[0, P], [Lq, nh], [1, Lq]])
                eng.dma_start(out=bstacks[bi][:], in_=bc_in)
                for h in range(h0, h1):
                    c, hf = h // 2, h % 2
                    sl = ct[64 * hf:64 * hf + 64, c, :]
                    bsl = bstacks[bi][64 * hf:64 * hf + 64, h - h0, :]
                    # b0 muls ride the idle gpsimd engine; b1's gate partA
                    # so keep them on the faster DVE
                    meng = nc.gpsimd if bi == 0 else nc.vector
                    meng.tensor_mul(sl, sl, bsl)

            # --- main loop over e-chunks (2 heads each) ----------------
            v2s = [None, None]   # v2 tiles by chunk parity
            pts = [[None, None], [None, None]]  # pt tiles [c%2][hf]
            pending = []
            for c in range(NCH):
                # output-proj weights trickle in behind the critical inputs
                nc.gpsimd.dma_start(out=obs[:, c, :], in_=OB[c * P:(c + 1) * P, :])
                # fused q/k projection for both heads of this chunk
                qkt2 = qkT.tile([P, Lq + Lk], BF16, tag="qkt2")
                for s0, s1 in _chunks(Lq + Lk):
                    ps = pssmall.tile([P, 512], F32, tag="small")
                    if s1 <= Lq:
                        nc.tensor.matmul(
                            out=ps[:, 0:s1 - s0], lhsT=w2s[:, c, :],
                            rhs=qts[:, c, s0:s1], start=True, stop=True)
                    elif s0 >= Lq:
                        nc.tensor.matmul(
                            out=ps[:, 0:s1 - s0], lhsT=w2s[:, c, :],
                            rhs=kts[:, c, s0 - Lq:s1 - Lq], start=True, stop=True)
                    else:
                        mid = Lq - s0
                        nc.tensor.matmul(
                            out=ps[:, 0:mid], lhsT=w2s[:, c, :],
                            rhs=qts[:, c, s0:Lq], start=True, stop=True)
                        nc.tensor.matmul(
                            out=ps[:, mid:s1 - s0], lhsT=w2s[:, c, :],
                            rhs=kts[:, c, 0:s1 - Lq], start=True, stop=True)
                    nc.vector.tensor_copy(qkt2[:, s0:s1], ps[:, 0:s1 - s0])
                qt2 = qkt2[:, 0:Lq]
                kt2 = qkt2[:, Lq:Lq + Lk]

                # v projection: k-tiles in 512-wide PSUM groups, evacuated
                # with strided APs into the [65,2] interleaved v2 layout
                v2 = vaug.tile([P, ntk, 130], BF16)
                v2s[c % 2] = v2
                for t0, t1 in _chunks(ntk * P):
                    ps = pssmall.tile([P, 512], F32, tag="small")
                    for t in range(t0 // P, (t1 + P - 1) // P):
                        nc.tensor.matmul(
                            out=ps[:, t * P - t0:(t + 1) * P - t0],
                            lhsT=vts[:, c, t * P:(t + 1) * P],
                            rhs=w2s[:, c, :],
                            start=True, stop=True,
                        )
                    nt = (t1 - t0) // P
                    base = v2[:, t0 // P, 0:64]
                    vt_out = bass.AP(
                        tensor=base.tensor, offset=base.offset,
                        ap=[list(base.ap[0]), [130, nt], [65, 2], [1, 64]])
                    nc.vector.tensor_copy(
                        vt_out,
                        ps[:, 0:t1 - t0].rearrange(
                            "p (nt two d) -> p nt two d", nt=nt, two=2))
                # denominator "ones" columns = slot-validity mask
                nc.gpsimd.tensor_copy(v2[:, :, 64], kms[:, :])
                nc.gpsimd.tensor_copy(v2[:, :, 129], kms[:, :])

                for hf in range(2):
                    h = 2 * c + hf
                    hq = qt2[64 * hf:64 * hf + 64, :]
                    hk = kt2[64 * hf:64 * hf + 64, :]
                    # scores (transposed, [k, q]) in 2-bank tiles + merged exp
                    pt = ppool.tile([P, ntk, Lq], BF16, tag=f"pt{hf}")
                    pts[c % 2][hf] = pt
                    for t0 in range(0, ntk, 2):
                        t1 = min(t0 + 2, ntk)
                        sps = psbig.tile([P, 1024], F32, tag="big")
                        for t in range(t0, t1):
                            for s0, s1 in _chunks(Lq):
                                nc.tensor.matmul(
                                    out=sps[:, (t - t0) * Lq + s0:(t - t0) * Lq + s1],
                                    lhsT=hk[:, t * P:(t + 1) * P],
                                    rhs=hq[:, s0:s1],
                                    start=True, stop=True,
                                )
                        w = (t1 - t0) * Lq
                        nc.scalar.activation(
                            out=pt[:, t0:t1, :], in_=sps[:, 0:w],
                            func=mybir.ActivationFunctionType.Exp,
                            scale=0.125,
                        )
                    # software pipeline: PV runs two heads behind scores so
                    # it never waits on the (slower) ACT exp stream
                    pending.append(h)
                    if len(pending) > 2:
                        ph = pending.pop(0)
                        emit_pv(ph)
                        for bi, (_, b1_, _) in enumerate(NB[:2]):
                            if ph == b1_ - 1:
                                emit_norm_batch(bi)

            for ph in pending:
                emit_pv(ph)
                for bi, (_, b1_, _) in enumerate(NB[:2]):
                    if ph == b1_ - 1:
                        emit_norm_batch(bi)
            # ship the unnormalized last chunk to the host for correction
            nc.sync.dma_start(out=CT7[:], in_=ct[:, NCH - 1, :])

            # output projection over chunks 0..NCH-2 (all device-normalized;
            # the host adds the last chunk's term from CT7/DEN).  Per-MM
            # subtile deps let early chunks' matmuls fill PE gaps during
            # the last heads' ACT-paced attention.
            for t in range(ntq):
                ya = psbig.tile([P, 1024], F32, tag="big")
                for c in range(NCH - 1):
                    for eh in range(2):
                        nc.tensor.matmul(
                            out=ya[:, 512 * eh:512 * (eh + 1)],
                            lhsT=ct[:, c, t * P:(t + 1) * P],
                            rhs=obs[:, c, 512 * eh:512 * (eh + 1)],
                            start=(c == 0), stop=(c == NCH - 2),
                        )
                ys = ystage.tile([P, E], BF16, tag="ys")
                if t % 2 == 0:
                    nc.scalar.copy(ys[:], ya[:])
                else:
                    nc.vector.tensor_copy(ys[:], ya[:])
                yeng = nc.sync if t % 2 == 0 else nc.gpsimd
                yeng.dma_start(out=Y[t * P:(t + 1) * P, :], in_=ys[:])

    nc.compile()
    return nc


def make_core_inputs(Q, K, V, HeadLinear, OutputLiner, QMask, KMask):
    """Host-side sharding/compaction (see module docstring)."""
    bf16 = ml_dtypes.bfloat16
    qm = np.asarray(QMask).astype(bool)
    km = np.asarray(KMask).astype(bool)
    qidxs = [np.nonzero(qm[b])[0] for b in range(B)]
    kidxs = [np.nonzero(km[b])[0] for b in range(B)]
    maxq = max(len(ix) for ix in qidxs)
    qcap = maxq
    if maxq > 512 and maxq % 512 <= 64:
        qcap = (maxq // 512) * 512
    qidxs = [ix[:qcap] for ix in qidxs]
    ntq = max(1, math.ceil(max(len(ix) for ix in qidxs) / P))
    ntk = max(1, math.ceil(max(len(ix) for ix in kidxs) / P))
    Lq, Lk = ntq * P, ntk * P

    w2 = np.zeros((P, NCH, P), dtype=np.float32)
    hl = np.asarray(HeadLinear, dtype=np.float32)
    for c in range(NCH):
        w2[0:64, c, 0:64] = hl[2 * c]
        w2[64:128, c, 64:128] = hl[2 * c + 1]
    w2b = w2.astype(bf16)
    ob = np.asarray(OutputLiner, dtype=np.float32).astype(bf16)

    in_maps = []
    for b in range(B):
        qi, ki = qidxs[b], kidxs[b]
        qkv = np.zeros((Lq + 2 * Lk, E), dtype=np.float32)
        qkv[:len(qi)] = np.asarray(Q[b], dtype=np.float32)[qi]
        qkv[Lq:Lq + len(ki)] = np.asarray(K[b], dtype=np.float32)[ki]
        qkv[Lq + Lk:Lq + Lk + len(ki)] = np.asarray(V[b], dtype=np.float32)[ki]
        kmc = np.zeros(Lk, dtype=np.float32)
        kmc[:len(ki)] = 1.0
        in_maps.append({
            "QKV": np.ascontiguousarray(qkv.T.astype(bf16)),
            "W2": w2b, "OB": ob,
            "KM": np.ascontiguousarray(kmc.reshape(ntk, P).T),
        })
    return in_maps, qidxs, ntq, ntk


_NC_CACHE = {}


def _get_nc(ntq, ntk):
    if (ntq, ntk) not in _NC_CACHE:
        _NC_CACHE[(ntq, ntk)] = build_bass(ntq, ntk)
    return _NC_CACHE[(ntq, ntk)]


def _host_tail(Q, K, V, HeadLinear, OutputLiner, KMask, b, tidx):
    """Exact fp32 attention for a few overflow queries of batch b."""
    hl = np.asarray(HeadLinear, dtype=np.float32)
    ob = np.asarray(OutputLiner, dtype=np.float32)
    ki = np.nonzero(np.asarray(KMask[b]).astype(bool))[0]
    q = np.asarray(Q[b], dtype=np.float32)[tidx]
    kk = np.asarray(K[b], dtype=np.float32)[ki]
    vv = np.asarray(V[b], dtype=np.float32)[ki]
    outs = []
    for h in range(H):
        sl = slice(h * D, (h + 1) * D)
        qh = q[:, sl] @ hl[h]
        kh = kk[:, sl] @ hl[h]
        vh = vv[:, sl] @ hl[h]
        s = (qh @ kh.T) / np.float32(np.sqrt(D))
        s -= s.max(axis=1, keepdims=True)
        p = np.exp(s)
        p /= p.sum(axis=1, keepdims=True)
        outs.append(p @ vh)
    return np.concatenate(outs, axis=1) @ ob


def kernel(Q, K, V, HeadLinear, OutputLiner, QMask, KMask):
    from concourse.bass_utils import run_bass_kernel_spmd

    in_maps, qidxs, ntq, ntk = make_core_inputs(
        Q, K, V, HeadLinear, OutputLiner, QMask, KMask)
    nc = _get_nc(ntq, ntk)
    res = run_bass_kernel_spmd(nc, in_maps, list(range(B)))
    out = np.zeros((B, L, E), dtype=np.float32)
    qm = np.asarray(QMask).astype(bool)
    bf16 = ml_dtypes.bfloat16
    # last-chunk normalize correction: the device used UNNORMALIZED ct
    # for the final e-chunk's contribution; add (ctN - ctU)^T @ OB_rows
    ob7 = np.asarray(OutputLiner, dtype=np.float32).astype(bf16).astype(
        np.float32)[(NCH - 1) * P:, :]
    for b in range(B):
        yc = np.asarray(res.results[b]["Y"]).astype(np.float32)
        ct7 = np.asarray(res.results[b]["CT7"]).astype(np.float32)
        den = np.asarray(res.results[b]["DEN"]).astype(np.float32)
        scale = np.repeat(1.0 / den, 64, axis=0)            # [128, Lq]
        yc = yc + (ct7 * scale).T @ ob7
        out[b][qidxs[b]] = yc[:len(qidxs[b])]
        full = np.nonzero(qm[b])[0]
        tidx = full[len(qidxs[b]):]
        if len(tidx):
            out[b][tidx] = _host_tail(
                Q, K, V, HeadLinear, OutputLiner, KMask, b, tidx)
    return out


# revision 40
# speedup vs baseline: 1.1532x; 1.1464x over previous
"""Trainium2 Bass kernel for the nn_MultiHeadAttention problem.

Data-parallel over batch: each of the 8 NeuronCores processes one batch
element independently (no collectives).

Mask compaction: the host gathers only the valid query/key positions
(QMask/KMask true), padded to a multiple of 128, and scatters the
output back (masked query rows are exactly zero in the reference).
With ~50% random masks this cuts the attention work ~4x.  If the max
query count only slightly exceeds a 512 multiple, the device is capped
there and the few overflow queries are computed exactly on the host.

V2 engine balance (per core, Lq=512, Lk=640, H=16 heads):
  PE     ~54us: proj + scores + PV + out-proj, software-pipelined so it
         streams continuously (HAM stays warm at 2.4 GHz).
  ACT    ~48us: exp ONLY, merged into 3 ACTIVATEs per head over 2-bank
         [128,1024] PSUM reads to amortize the ~293ns/instr overhead.
  DVE    ~50us: all PSUM evacuation (proj/vproj/pv/den), batched recip,
         batch-1/2 normalize muls, ysum copies/adds.
  GPSIMD: v2 ones-column copies, batch-0 normalize muls, some DMA
         triggers (obs, rbounce/bcs b0, den h<8).
  Sync:  input chunk DMA triggers, den h>=8, Y out.

Normalize: denominator rows DMA-transposed into dstacks, batched
reciprocal on DVE; recip broadcast across partitions via one batched
DRAM-bounce DMA per batch (b0, b1); the final batch (last 2 heads) uses
tiny PE one-hot matmuls into PSUM instead, killing the DMA round-trip
latency on the tail.  Y is written bf16 per q-tile (host casts to f32).
"""

import math
import os
import sys

import numpy as np

try:
    import concourse  # noqa: F401
except ImportError:  # pragma: no cover
    for _p in ("/opt/trn_rl_repo", os.path.expanduser("~/.axon_site/_ro/trn_rl_repo")):
        if os.path.isdir(_p) and _p not in sys.path:
            sys.path.insert(0, _p)

import ml_dtypes

import concourse.bass as bass
import concourse.tile as tile
from concourse import bacc, mybir

B, L, E, H, D = 8, 1024, 1024, 16, 64
P = 128          # partitions
NCH = E // P     # 8 e-chunks (2 heads each)
F32 = mybir.dt.float32
BF16 = mybir.dt.bfloat16


def _chunks(n, step=512):
    return [(s, min(s + step, n)) for s in range(0, n, step)]


def build_bass(ntq, ntk):
    Lq, Lk = ntq * P, ntk * P
    nc = bacc.Bacc(None, target_bir_lowering=False, debug=False)

    Lqkv = Lq + 2 * Lk
    QKV = nc.declare_dram_parameter("QKV", [E, Lqkv], BF16, isOutput=False)
    W2 = nc.declare_dram_parameter("W2", [P, NCH, P], BF16, isOutput=False)
    OB = nc.declare_dram_parameter("OB", [E, E], BF16, isOutput=False)
    KM = nc.declare_dram_parameter("KM", [P, ntk], F32, isOutput=False)
    Y = nc.declare_dram_parameter("Y", [Lq, E], BF16, isOutput=True)
    CT7 = nc.declare_dram_parameter("CT7", [P, Lq], BF16, isOutput=True)
    DEN = nc.declare_dram_parameter("DEN", [2, Lq], F32, isOutput=True)
    rbounce = nc.dram_tensor("rbounce", [H, Lq], BF16)

    # normalize batches: heads [h0,h1) of batches 0/1 are normalized on
    # device once their PVs land; the final 2 heads stay UNNORMALIZED on
    # device (ct7/dens shipped out, the host applies the exact rank-128
    # correction to Y), killing the tail's normalize latency chain.
    NB = [(0, H // 2, None), (H // 2, H - 2, None), (H - 2, H, None)]

    with tile.TileContext(nc) as tc:
        with (
            tc.tile_pool(name="singles", bufs=1) as singles,
            tc.tile_pool(name="qkT", bufs=2) as qkT,
            tc.tile_pool(name="vaug", bufs=2) as vaug,
            tc.tile_pool(name="ppool", bufs=2) as ppool,
            tc.tile_pool(name="ystage", bufs=4) as ystage,
            tc.tile_pool(name="dtpool", bufs=2) as dtpool,
            tc.tile_pool(name="psbig", bufs=2, space="PSUM") as psbig,
            tc.tile_pool(name="pspv", bufs=2, space="PSUM") as pspv,
            tc.tile_pool(name="pssmall", bufs=2, space="PSUM") as pssmall,
        ):
            # --- persistent SBUF tensors -------------------------------
            qkvts = singles.tile([P, NCH, Lqkv], BF16)
            qts = qkvts[:, :, 0:Lq]
            kts = qkvts[:, :, Lq:Lq + Lk]
            vts = qkvts[:, :, Lq + Lk:Lqkv]
            obs = singles.tile([P, NCH, E], BF16)
            w2s = singles.tile([P, NCH, P], BF16)
            kms = singles.tile([P, ntk], F32)
            ct = singles.tile([P, NCH, Lq], BF16)
            dstacks, rstacks, bstacks = [], [], []
            for bi, (h0, h1, _) in enumerate(NB[:2]):
                nh = h1 - h0
                ds = singles.tile([nh * ntq, P], F32, tag=f"ds{bi}")
                rs = singles.tile([nh * ntq, P], BF16, tag=f"rs{bi}")
                dstacks.append(ds)
                rstacks.append(rs)
                bs = singles.tile([P, nh, Lq], BF16, tag=f"bs{bi}")
                bstacks.append(bs)

            # --- table load + input DMAs -------------------------------
            # dummy exp first so the ~2.7us ACT table load overlaps DMAs
            dume = singles.tile([P, 8], BF16)
            nc.vector.memset(dume[:], 0.0)
            nc.scalar.activation(out=dume[:], in_=dume[:],
                                 func=mybir.ActivationFunctionType.Exp)
            nc.gpsimd.dma_start(out=kms[:], in_=KM[:])
            # PE warmup: cold dummy matmuls so the HAM clock gate opens
            # while the first chunk DMAs land (~3us of cold N=256 MMs)
            warm = singles.tile([P, 256], BF16)
            nc.vector.memset(warm[:], 0.0)
            for wi in range(14):
                wps = pssmall.tile([P, 512], F32, tag="small")
                nc.tensor.matmul(out=wps[:, 0:256], lhsT=warm[:, 0:128],
                                 rhs=warm[:], start=True, stop=True)
            # inputs as small per-chunk pieces in COMPUTE order across two
            # queues (a single queue moves only ~130 GB/s, and chunk 0 must
            # land first): sync carries w2-chunk + q + k, gpsimd carries v.
            for c in range(NCH):
                nc.sync.dma_start(out=w2s[:, c, :],
                                  in_=W2[:, c, :])
                nc.sync.dma_start(out=qkvts[:, c, 0:Lq],
                                  in_=QKV[c * P:(c + 1) * P, 0:Lq])
                nc.sync.dma_start(out=qkvts[:, c, Lq:Lq + Lk],
                                  in_=QKV[c * P:(c + 1) * P, Lq:Lq + Lk])
                nc.gpsimd.dma_start(out=qkvts[:, c, Lq + Lk:Lqkv],
                                    in_=QKV[c * P:(c + 1) * P, Lq + Lk:Lqkv])
            # output-proj weights queue behind the v's (needed only late)
            for c in range(NCH):
                nc.gpsimd.dma_start(out=obs[:, c, :],
                                    in_=OB[c * P:(c + 1) * P, :])

            def batch_of(h):
                return next(i for i, (a, b2, _) in enumerate(NB) if a <= h < b2)

            def emit_pv(h):
                """PV for head h (pt/v2 already computed), plus evacuation."""
                c, hf = h // 2, h % 2
                pv = pspv.tile([65, Lq], F32, tag="pv")
                for kt in range(ntk):
                    nc.tensor.matmul(
                        out=pv[:],
                        lhsT=v2s[c % 2][:, kt, 65 * hf:65 * hf + 65],
                        rhs=pts[c % 2][hf][:, kt, :],
                        start=(kt == 0), stop=(kt == ntk - 1),
                    )
                # evacuate: unnormalized C^T rows + denominator row
                nc.vector.tensor_copy(ct[64 * hf:64 * hf + 64, c, :], pv[0:64, :])
                dtmp = dtpool.tile([1, Lq], F32)
                nc.vector.tensor_copy(dtmp[:], pv[64:65, :])
                bi = batch_of(h)
                hrel = h - NB[bi][0]
                if bi < 2:
                    eng = nc.gpsimd if h < H // 2 else nc.sync
                    eng.dma_start(
                        out=dstacks[bi][hrel * ntq:(hrel + 1) * ntq, :],
                        in_=dtmp[:])
                else:
                    # last 2 heads: denominator goes to the host
                    nc.sync.dma_start(out=DEN[hrel:hrel + 1, :], in_=dtmp[:])

            def emit_norm_batch(bi):
                h0, h1, _ = NB[bi]
                nh = h1 - h0
                with nc.allow_low_precision(reason="softmax recip bf16"):
                    nc.vector.reciprocal(out=rstacks[bi][:], in_=dstacks[bi][:])
                # DRAM bounce + ONE batched broadcast DMA for the batch
                eng = nc.gpsimd if bi == 0 else nc.sync
                eng.dma_start(out=rbounce[h0:h1, :], in_=rstacks[bi][:])
                src = rbounce[h0:h1, :]
                bc_in = bass.AP(
                    tensor=src.tensor, offset=src.offset,
                    ap=[[0, P], [Lq, nh], [1, Lq]])
                eng.dma_start(out=bstacks[bi][:], in_=bc_in)
                for h in range(h0, h1):
                    c, hf = h // 2, h % 2
                    sl = ct[64 * hf:64 * hf + 64, c, :]
                    bsl = bstacks[bi][64 * hf:64 * hf + 64, h - h0, :]
                    # b0 muls ride the idle gpsimd engine; b1's gate partA
                    # so keep them on the faster DVE
                    meng = nc.gpsimd if bi == 0 else nc.vector
                    meng.tensor_mul(sl, sl, bsl)

            # --- main loop over e-chunks (2 heads each) ----------------
            v2s = [None, None]   # v2 tiles by chunk parity
            pts = [[None, None], [None, None]]  # pt tiles [c%2][hf]
            pending = []
            for c in range(NCH):
                # fused q/k projection for both heads of this chunk
                qkt2 = qkT.tile([P, Lq + Lk], BF16, tag="qkt2")
                for s0, s1 in _chunks(Lq + Lk):
                    ps = pssmall.tile([P, 512], F32, tag="small")
                    if s1 <= Lq:
                        nc.tensor.matmul(
                            out=ps[:, 0:s1 - s0], lhsT=w2s[:, c, :],
                            rhs=qts[:, c, s0:s1], start=True, stop=True)
                    elif s0 >= Lq:
                        nc.tensor.matmul(
                            out=ps[:, 0:s1 - s0], lhsT=w2s[:, c, :],
                            rhs=kts[:, c, s0 - Lq:s1 - Lq], start=True, stop=True)
                    else:
                        mid = Lq - s0
                        nc.tensor.matmul(
                            out=ps[:, 0:mid], lhsT=w2s[:, c, :],
                            rhs=qts[:, c, s0:Lq], start=True, stop=True)
                        nc.tensor.matmul(
                            out=ps[:, mid:s1 - s0], lhsT=w2s[:, c, :],
                            rhs=kts[:, c, 0:s1 - Lq], start=True, stop=True)
                    nc.vector.tensor_copy(qkt2[:, s0:s1], ps[:, 0:s1 - s0])
                qt2 = qkt2[:, 0:Lq]
                kt2 = qkt2[:, Lq:Lq + Lk]

                # v projection: k-tiles in 512-wide PSUM groups, evacuated
                # with strided APs into the [65,2] interleaved v2 layout
                v2 = vaug.tile([P, ntk, 130], BF16)
                v2s[c % 2] = v2
                for t0, t1 in _chunks(ntk * P):
                    ps = pssmall.tile([P, 512], F32, tag="small")
                    for t in range(t0 // P, (t1 + P - 1) // P):
                        nc.tensor.matmul(
                            out=ps[:, t * P - t0:(t + 1) * P - t0],
                            lhsT=vts[:, c, t * P:(t + 1) * P],
                            rhs=w2s[:, c, :],
                            start=True, stop=True,
                        )
                    nt = (t1 - t0) // P
                    base = v2[:, t0 // P, 0:64]
                    vt_out = bass.AP(
                        tensor=base.tensor, offset=base.offset,
                        ap=[list(base.ap[0]), [130, nt], [65, 2], [1, 64]])
                    nc.vector.tensor_copy(
                        vt_out,
                        ps[:, 0:t1 - t0].rearrange(
                            "p (nt two d) -> p nt two d", nt=nt, two=2))
                # denominator "ones" columns = slot-validity mask
                nc.gpsimd.tensor_copy(v2[:, :, 64], kms[:, :])
                nc.gpsimd.tensor_copy(v2[:, :, 129], kms[:, :])

                for hf in range(2):
                    h = 2 * c + hf
                    hq = qt2[64 * hf:64 * hf + 64, :]
                    hk = kt2[64 * hf:64 * hf + 64, :]
                    # scores (transposed, [k, q]) in 2-bank tiles + merged exp
                    pt = ppool.tile([P, ntk, Lq], BF16, tag=f"pt{hf}")
                    pts[c % 2][hf] = pt
                    for t0 in range(0, ntk, 2):
                        t1 = min(t0 + 2, ntk)
                        sps = psbig.tile([P, 1024], F32, tag="big")
                        for t in range(t0, t1):
                            for s0, s1 in _chunks(Lq):
                                nc.tensor.matmul(
                                    out=sps[:, (t - t0) * Lq + s0:(t - t0) * Lq + s1],
                                    lhsT=hk[:, t * P:(t + 1) * P],
                                    rhs=hq[:, s0:s1],
                                    start=True, stop=True,
                                )
                        w = (t1 - t0) * Lq
                        nc.scalar.activation(
                            out=pt[:, t0:t1, :], in_=sps[:, 0:w],
                            func=mybir.ActivationFunctionType.Exp,
                            scale=0.125,
                        )
                    # software pipeline: PV runs two heads behind scores so
                    # it never waits on the (slower) ACT exp stream
                    pending.append(h)
                    if len(pending) > 2:
                        ph = pending.pop(0)
                        emit_pv(ph)
                        for bi, (_, b1_, _) in enumerate(NB[:2]):
                            if ph == b1_ - 1:
                                emit_norm_batch(bi)

            for ph in pending:
                emit_pv(ph)
                for bi, (_, b1_, _) in enumerate(NB[:2]):
                    if ph == b1_ - 1:
                        emit_norm_batch(bi)
            # ship the unnormalized last chunk to the host for correction
            nc.sync.dma_start(out=CT7[:], in_=ct[:, NCH - 1, :])

            # output projection over chunks 0..NCH-2 (all device-normalized;
            # the host adds the last chunk's term from CT7/DEN).  Per-MM
            # subtile deps let early chunks' matmuls fill PE gaps during
            # the last heads' ACT-paced attention.
            for t in range(ntq):
                ya = psbig.tile([P, 1024], F32, tag="big")
                for c in range(NCH - 1):
                    for eh in range(2):
                        nc.tensor.matmul(
                            out=ya[:, 512 * eh:512 * (eh + 1)],
                            lhsT=ct[:, c, t * P:(t + 1) * P],
                            rhs=obs[:, c, 512 * eh:512 * (eh + 1)],
                            start=(c == 0), stop=(c == NCH - 2),
                        )
                ys = ystage.tile([P, E], BF16, tag="ys")
                if t % 2 == 0:
                    nc.scalar.copy(ys[:], ya[:])
                else:
                    nc.vector.tensor_copy(ys[:], ya[:])
                yeng = nc.sync if t % 2 == 0 else nc.gpsimd
                yeng.dma_start(out=Y[t * P:(t + 1) * P, :], in_=ys[:])

    nc.compile()
    return nc


def make_core_inputs(Q, K, V, HeadLinear, OutputLiner, QMask, KMask):
    """Host-side sharding/compaction (see module docstring)."""
    bf16 = ml_dtypes.bfloat16
    qm = np.asarray(QMask).astype(bool)
    km = np.asarray(KMask).astype(bool)
    qidxs = [np.nonzero(qm[b])[0] for b in range(B)]
    kidxs = [np.nonzero(km[b])[0] for b in range(B)]
    maxq = max(len(ix) for ix in qidxs)
    qcap = maxq
    if maxq > 512 and maxq % 512 <= 64:
        qcap = (maxq // 512) * 512
    qidxs = [ix[:qcap] for ix in qidxs]
    ntq = max(1, math.ceil(max(len(ix) for ix in qidxs) / P))
    ntk = max(1, math.ceil(max(len(ix) for ix in kidxs) / P))
    Lq, Lk = ntq * P, ntk * P

    w2 = np.zeros((P, NCH, P), dtype=np.float32)
    hl = np.asarray(HeadLinear, dtype=np.float32)
    for c in range(NCH):
        w2[0:64, c, 0:64] = hl[2 * c]
        w2[64:128, c, 64:128] = hl[2 * c + 1]
    w2b = w2.astype(bf16)
    ob = np.asarray(OutputLiner, dtype=np.float32).astype(bf16)

    in_maps = []
    for b in range(B):
        qi, ki = qidxs[b], kidxs[b]
        qkv = np.zeros((Lq + 2 * Lk, E), dtype=np.float32)
        qkv[:len(qi)] = np.asarray(Q[b], dtype=np.float32)[qi]
        qkv[Lq:Lq + len(ki)] = np.asarray(K[b], dtype=np.float32)[ki]
        qkv[Lq + Lk:Lq + Lk + len(ki)] = np.asarray(V[b], dtype=np.float32)[ki]
        kmc = np.zeros(Lk, dtype=np.float32)
        kmc[:len(ki)] = 1.0
        in_maps.append({
            "QKV": np.ascontiguousarray(qkv.T.astype(bf16)),
            "W2": w2b, "OB": ob,
            "KM": np.ascontiguousarray(kmc.reshape(ntk, P).T),
        })
    return in_maps, qidxs, ntq, ntk


_NC_CACHE = {}


def _get_nc(ntq, ntk):
    if (ntq, ntk) not in _NC_CACHE:
        _NC_CACHE[(ntq, ntk)] = build_bass(ntq, ntk)
    return _NC_CACHE[(ntq, ntk)]


def _host_tail(Q, K, V, HeadLinear, OutputLiner, KMask, b, tidx):
    """Exact fp32 attention for a few overflow queries of batch b."""
    hl = np.asarray(HeadLinear, dtype=np.float32)
    ob = np.asarray(OutputLiner, dtype=np.float32)
    ki = np.nonzero(np.asarray(KMask[b]).astype(bool))[0]
    q = np.asarray(Q[b], dtype=np.float32)[tidx]
    kk = np.asarray(K[b], dtype=np.float32)[ki]
    vv = np.asarray(V[b], dtype=np.float32)[ki]
    outs = []
    for h in range(H):
        sl = slice(h * D, (h + 1) * D)
        qh = q[:, sl] @ hl[h]
        kh = kk[:, sl] @ hl[h]
        vh = vv[:, sl] @ hl[h]
        s = (qh @ kh.T) / np.float32(np.sqrt(D))
        s -= s.max(axis=1, keepdims=True)
        p = np.exp(s)
        p /= p.sum(axis=1, keepdims=True)
        outs.append(p @ vh)
    return np.concatenate(outs, axis=1) @ ob


def kernel(Q, K, V, HeadLinear, OutputLiner, QMask, KMask):
    from concourse.bass_utils import run_bass_kernel_spmd

    in_maps, qidxs, ntq, ntk = make_core_inputs(
        Q, K, V, HeadLinear, OutputLiner, QMask, KMask)
    nc = _get_nc(ntq, ntk)
    res = run_bass_kernel_spmd(nc, in_maps, list(range(B)))
    out = np.zeros((B, L, E), dtype=np.float32)
    qm = np.asarray(QMask).astype(bool)
    bf16 = ml_dtypes.bfloat16
    # last-chunk normalize correction: the device used UNNORMALIZED ct
    # for the final e-chunk's contribution; add (ctN - ctU)^T @ OB_rows
    ob7 = np.asarray(OutputLiner, dtype=np.float32).astype(bf16).astype(
        np.float32)[(NCH - 1) * P:, :]
    for b in range(B):
        yc = np.asarray(res.results[b]["Y"]).astype(np.float32)
        ct7 = np.asarray(res.results[b]["CT7"]).astype(np.float32)
        den = np.asarray(res.results[b]["DEN"]).astype(np.float32)
        scale = np.repeat(1.0 / den, 64, axis=0)            # [128, Lq]
        yc = yc + (ct7 * scale).T @ ob7
        out[b][qidxs[b]] = yc[:len(qidxs[b])]
        full = np.nonzero(qm[b])[0]
        tidx = full[len(qidxs[b]):]
        if len(tidx):
            out[b][tidx] = _host_tail(
                Q, K, V, HeadLinear, OutputLiner, KMask, b, tidx)
    return out


# revision 46
# speedup vs baseline: 1.2590x; 1.0918x over previous
"""Trainium2 Bass kernel for the nn_MultiHeadAttention problem.

Data-parallel over batch: each of the 8 NeuronCores processes one batch
element independently (no collectives).

Mask compaction: the host gathers only the valid query/key positions
(QMask/KMask true), padded to a multiple of 128, and scatters the
output back (masked query rows are exactly zero in the reference).
With ~50% random masks this cuts the attention work ~4x.  If the max
query count only slightly exceeds a 512 multiple, the device is capped
there and the few overflow queries are computed exactly on the host.

V2 engine balance (per core, Lq=512, Lk=640, H=16 heads):
  PE     ~54us: proj + scores + PV + out-proj, software-pipelined so it
         streams continuously (HAM stays warm at 2.4 GHz).
  ACT    ~48us: exp ONLY, merged into 3 ACTIVATEs per head over 2-bank
         [128,1024] PSUM reads to amortize the ~293ns/instr overhead.
  DVE    ~50us: all PSUM evacuation (proj/vproj/pv/den), batched recip,
         batch-1/2 normalize muls, ysum copies/adds.
  GPSIMD: v2 ones-column copies, batch-0 normalize muls, some DMA
         triggers (obs, rbounce/bcs b0, den h<8).
  Sync:  input chunk DMA triggers, den h>=8, Y out.

Normalize: denominator rows DMA-transposed into dstacks, batched
reciprocal on DVE; recip broadcast across partitions via one batched
DRAM-bounce DMA per batch (b0, b1); the final batch (last 2 heads) uses
tiny PE one-hot matmuls into PSUM instead, killing the DMA round-trip
latency on the tail.  Y is written bf16 per q-tile (host casts to f32).
"""

import math
import os
import sys

import numpy as np

try:
    import concourse  # noqa: F401
except ImportError:  # pragma: no cover
    for _p in ("/opt/trn_rl_repo", os.path.expanduser("~/.axon_site/_ro/trn_rl_repo")):
        if os.path.isdir(_p) and _p not in sys.path:
            sys.path.insert(0, _p)

import ml_dtypes

import concourse.bass as bass
import concourse.tile as tile
from concourse import bacc, mybir

B, L, E, H, D = 8, 1024, 1024, 16, 64
P = 128          # partitions
NCH = E // P     # 8 e-chunks (2 heads each)
F32 = mybir.dt.float32
BF16 = mybir.dt.bfloat16


def _chunks(n, step=512):
    return [(s, min(s + step, n)) for s in range(0, n, step)]


def build_bass(ntq, ntk):
    Lq, Lk = ntq * P, ntk * P
    nc = bacc.Bacc(None, target_bir_lowering=False, debug=False)

    Lqkv = Lq + 2 * Lk
    QKV = nc.declare_dram_parameter("QKV", [E, Lqkv], BF16, isOutput=False)
    W2 = nc.declare_dram_parameter("W2", [P, NCH, P], BF16, isOutput=False)
    OB = nc.declare_dram_parameter("OB", [E, E], BF16, isOutput=False)
    KM = nc.declare_dram_parameter("KM", [P, ntk], F32, isOutput=False)
    Y = nc.declare_dram_parameter("Y", [Lq, E], BF16, isOutput=True)
    CT7 = nc.declare_dram_parameter("CT7", [P, Lq], BF16, isOutput=True)
    DEN = nc.declare_dram_parameter("DEN", [2, Lq], F32, isOutput=True)
    rbounce = nc.dram_tensor("rbounce", [H, Lq], BF16)

    # normalize batches: heads [h0,h1) of the first batches are normalized
    # on device once their PVs land (the last one kept small so partA's
    # gate closes early); the final 2 heads stay UNNORMALIZED on device
    # (ct7/dens shipped out, the host applies the exact rank-128
    # correction to Y), killing the tail's normalize latency chain.
    NB = [(0, 8, None), (8, 12, None), (12, 14, None), (H - 2, H, None)]

    with tile.TileContext(nc) as tc:
        with (
            tc.tile_pool(name="singles", bufs=1) as singles,
            tc.tile_pool(name="qkT", bufs=2) as qkT,
            tc.tile_pool(name="vaug", bufs=2) as vaug,
            tc.tile_pool(name="ppool", bufs=2) as ppool,
            tc.tile_pool(name="ystage", bufs=4) as ystage,
            tc.tile_pool(name="dtpool", bufs=2) as dtpool,
            tc.tile_pool(name="psbig", bufs=2, space="PSUM") as psbig,
            tc.tile_pool(name="pspv", bufs=2, space="PSUM") as pspv,
            tc.tile_pool(name="pssmall", bufs=2, space="PSUM") as pssmall,
        ):
            # --- persistent SBUF tensors -------------------------------
            qkvts = singles.tile([P, NCH, Lqkv], BF16)
            qts = qkvts[:, :, 0:Lq]
            kts = qkvts[:, :, Lq:Lq + Lk]
            vts = qkvts[:, :, Lq + Lk:Lqkv]
            obs = singles.tile([P, NCH, E], BF16)
            w2s = singles.tile([P, NCH, P], BF16)
            kms = singles.tile([P, ntk], F32)
            ct = singles.tile([P, NCH, Lq], BF16)
            dstacks, rstacks, bstacks = [], [], []
            for bi, (h0, h1, _) in enumerate(NB[:3]):
                nh = h1 - h0
                ds = singles.tile([nh * ntq, P], F32, tag=f"ds{bi}")
                rs = singles.tile([nh * ntq, P], BF16, tag=f"rs{bi}")
                dstacks.append(ds)
                rstacks.append(rs)
                bs = singles.tile([P, nh, Lq], BF16, tag=f"bs{bi}")
                bstacks.append(bs)

            # --- table load + input DMAs -------------------------------
            # dummy exp first so the ~2.7us ACT table load overlaps DMAs
            dume = singles.tile([P, 8], BF16)
            nc.vector.memset(dume[:], 0.0)
            nc.scalar.activation(out=dume[:], in_=dume[:],
                                 func=mybir.ActivationFunctionType.Exp)
            nc.gpsimd.dma_start(out=kms[:], in_=KM[:])
            # PE warmup: cold dummy matmuls so the HAM clock gate opens
            # while the first chunk DMAs land (~3us of cold N=256 MMs)
            warm = singles.tile([P, 256], BF16)
            nc.vector.memset(warm[:], 0.0)
            for wi in range(14):
                wps = pssmall.tile([P, 512], F32, tag="small")
                nc.tensor.matmul(out=wps[:, 0:256], lhsT=warm[:, 0:128],
                                 rhs=warm[:], start=True, stop=True)
            # inputs as small per-chunk pieces in COMPUTE order across two
            # queues (a single queue moves only ~130 GB/s, and chunk 0 must
            # land first): sync carries w2-chunk + q + k, gpsimd carries v.
            for c in range(NCH):
                nc.sync.dma_start(out=w2s[:, c, :],
                                  in_=W2[:, c, :])
                nc.sync.dma_start(out=qkvts[:, c, 0:Lq],
                                  in_=QKV[c * P:(c + 1) * P, 0:Lq])
                nc.sync.dma_start(out=qkvts[:, c, Lq:Lq + Lk],
                                  in_=QKV[c * P:(c + 1) * P, Lq:Lq + Lk])
                nc.gpsimd.dma_start(out=qkvts[:, c, Lq + Lk:Lqkv],
                                    in_=QKV[c * P:(c + 1) * P, Lq + Lk:Lqkv])
            # output-proj weights queue behind the v's (needed only late)
            for c in range(NCH):
                nc.gpsimd.dma_start(out=obs[:, c, :],
                                    in_=OB[c * P:(c + 1) * P, :])

            def batch_of(h):
                return next(i for i, (a, b2, _) in enumerate(NB) if a <= h < b2)

            def emit_pv(h):
                """PV for head h (pt/v2 already computed), plus evacuation."""
                c, hf = h // 2, h % 2
                pv = pspv.tile([65, Lq], F32, tag="pv")
                for kt in range(ntk):
                    nc.tensor.matmul(
                        out=pv[:],
                        lhsT=v2s[c % 2][:, kt, 65 * hf:65 * hf + 65],
                        rhs=pts[c % 2][hf][:, kt, :],
                        start=(kt == 0), stop=(kt == ntk - 1),
                    )
                # evacuate: unnormalized C^T rows + denominator row
                nc.vector.tensor_copy(ct[64 * hf:64 * hf + 64, c, :], pv[0:64, :])
                dtmp = dtpool.tile([1, Lq], F32)
                nc.vector.tensor_copy(dtmp[:], pv[64:65, :])
                bi = batch_of(h)
                hrel = h - NB[bi][0]
                if bi < 3:
                    eng = nc.gpsimd if h < H // 2 else nc.sync
                    eng.dma_start(
                        out=dstacks[bi][hrel * ntq:(hrel + 1) * ntq, :],
                        in_=dtmp[:])
                else:
                    # last 2 heads: denominator goes to the host
                    nc.sync.dma_start(out=DEN[hrel:hrel + 1, :], in_=dtmp[:])

            def emit_norm_batch(bi):
                h0, h1, _ = NB[bi]
                nh = h1 - h0
                with nc.allow_low_precision(reason="softmax recip bf16"):
                    nc.vector.reciprocal(out=rstacks[bi][:], in_=dstacks[bi][:])
                # DRAM bounce + ONE batched broadcast DMA for the batch
                eng = nc.gpsimd if bi == 0 else nc.sync
                eng.dma_start(out=rbounce[h0:h1, :], in_=rstacks[bi][:])
                src = rbounce[h0:h1, :]
                bc_in = bass.AP(
                    tensor=src.tensor, offset=src.offset,
                    ap=[[0, P], [Lq, nh], [1, Lq]])
                eng.dma_start(out=bstacks[bi][:], in_=bc_in)
                for h in range(h0, h1):
                    c, hf = h // 2, h % 2
                    sl = ct[64 * hf:64 * hf + 64, c, :]
                    bsl = bstacks[bi][64 * hf:64 * hf + 64, h - h0, :]
                    # early batches ride the idle gpsimd engine; the last
                    # one gates partA so keep it on the faster DVE
                    meng = nc.gpsimd if bi < 2 else nc.vector
                    meng.tensor_mul(sl, sl, bsl)

            # --- main loop over e-chunks (2 heads each) ----------------
            v2s = [None, None]   # v2 tiles by chunk parity
            pts = [[None, None], [None, None]]  # pt tiles [c%2][hf]
            pending = []
            for c in range(NCH):
                # fused q/k projection for both heads of this chunk
                qkt2 = qkT.tile([P, Lq + Lk], BF16, tag="qkt2")
                for s0, s1 in _chunks(Lq + Lk):
                    ps = pssmall.tile([P, 512], F32, tag="small")
                    if s1 <= Lq:
                        nc.tensor.matmul(
                            out=ps[:, 0:s1 - s0], lhsT=w2s[:, c, :],
                            rhs=qts[:, c, s0:s1], start=True, stop=True)
                    elif s0 >= Lq:
                        nc.tensor.matmul(
                            out=ps[:, 0:s1 - s0], lhsT=w2s[:, c, :],
                            rhs=kts[:, c, s0 - Lq:s1 - Lq], start=True, stop=True)
                    else:
                        mid = Lq - s0
                        nc.tensor.matmul(
                            out=ps[:, 0:mid], lhsT=w2s[:, c, :],
                            rhs=qts[:, c, s0:Lq], start=True, stop=True)
                        nc.tensor.matmul(
                            out=ps[:, mid:s1 - s0], lhsT=w2s[:, c, :],
                            rhs=kts[:, c, 0:s1 - Lq], start=True, stop=True)
                    nc.vector.tensor_copy(qkt2[:, s0:s1], ps[:, 0:s1 - s0])
                qt2 = qkt2[:, 0:Lq]
                kt2 = qkt2[:, Lq:Lq + Lk]

                # v projection: k-tiles in 512-wide PSUM groups, evacuated
                # with strided APs into the [65,2] interleaved v2 layout
                v2 = vaug.tile([P, ntk, 130], BF16)
                v2s[c % 2] = v2
                for t0, t1 in _chunks(ntk * P):
                    ps = pssmall.tile([P, 512], F32, tag="small")
                    for t in range(t0 // P, (t1 + P - 1) // P):
                        nc.tensor.matmul(
                            out=ps[:, t * P - t0:(t + 1) * P - t0],
                            lhsT=vts[:, c, t * P:(t + 1) * P],
                            rhs=w2s[:, c, :],
                            start=True, stop=True,
                        )
                    nt = (t1 - t0) // P
                    base = v2[:, t0 // P, 0:64]
                    vt_out = bass.AP(
                        tensor=base.tensor, offset=base.offset,
                        ap=[list(base.ap[0]), [130, nt], [65, 2], [1, 64]])
                    nc.vector.tensor_copy(
                        vt_out,
                        ps[:, 0:t1 - t0].rearrange(
                            "p (nt two d) -> p nt two d", nt=nt, two=2))
                # denominator "ones" columns = slot-validity mask
                nc.gpsimd.tensor_copy(v2[:, :, 64], kms[:, :])
                nc.gpsimd.tensor_copy(v2[:, :, 129], kms[:, :])

                for hf in range(2):
                    h = 2 * c + hf
                    hq = qt2[64 * hf:64 * hf + 64, :]
                    hk = kt2[64 * hf:64 * hf + 64, :]
                    # scores (transposed, [k, q]) in 2-bank tiles + merged exp
                    pt = ppool.tile([P, ntk, Lq], BF16, tag=f"pt{hf}")
                    pts[c % 2][hf] = pt
                    for t0 in range(0, ntk, 2):
                        t1 = min(t0 + 2, ntk)
                        sps = psbig.tile([P, 1024], F32, tag="big")
                        for t in range(t0, t1):
                            for s0, s1 in _chunks(Lq):
                                nc.tensor.matmul(
                                    out=sps[:, (t - t0) * Lq + s0:(t - t0) * Lq + s1],
                                    lhsT=hk[:, t * P:(t + 1) * P],
                                    rhs=hq[:, s0:s1],
                                    start=True, stop=True,
                                )
                        w = (t1 - t0) * Lq
                        nc.scalar.activation(
                            out=pt[:, t0:t1, :], in_=sps[:, 0:w],
                            func=mybir.ActivationFunctionType.Exp,
                            scale=0.125,
                        )
                    # software pipeline: PV runs one head behind scores
                    pending.append(h)
                    if len(pending) > 1:
                        ph = pending.pop(0)
                        emit_pv(ph)
                        for bi, (_, b1_, _) in enumerate(NB[:3]):
                            if ph == b1_ - 1:
                                emit_norm_batch(bi)

            for ph in pending:
                emit_pv(ph)
                for bi, (_, b1_, _) in enumerate(NB[:3]):
                    if ph == b1_ - 1:
                        emit_norm_batch(bi)
            # ship the unnormalized last chunk to the host for correction
            nc.sync.dma_start(out=CT7[:], in_=ct[:, NCH - 1, :])

            # output projection over chunks 0..NCH-2 (all device-normalized;
            # the host adds the last chunk's term from CT7/DEN).  Per-MM
            # subtile deps let early chunks' matmuls fill PE gaps during
            # the last heads' ACT-paced attention.
            for t in range(ntq):
                ya0 = pssmall.tile([P, 512], F32, tag="small")
                ya1 = pssmall.tile([P, 512], F32, tag="small")
                for c in range(NCH - 1):
                    for eh in range(2):
                        nc.tensor.matmul(
                            out=[ya0, ya1][eh][:],
                            lhsT=ct[:, c, t * P:(t + 1) * P],
                            rhs=obs[:, c, 512 * eh:512 * (eh + 1)],
                            start=(c == 0), stop=(c == NCH - 2),
                        )
                ys = ystage.tile([P, E], BF16, tag="ys")
                if t % 2 == 0:
                    nc.scalar.copy(ys[:, 0:512], ya0[:])
                    nc.vector.tensor_copy(ys[:, 512:1024], ya1[:])
                else:
                    nc.vector.tensor_copy(ys[:, 0:512], ya0[:])
                    nc.scalar.copy(ys[:, 512:1024], ya1[:])
                yeng = nc.sync if t % 2 == 0 else nc.gpsimd
                yeng.dma_start(out=Y[t * P:(t + 1) * P, :], in_=ys[:])

    nc.compile()
    return nc


def make_core_inputs(Q, K, V, HeadLinear, OutputLiner, QMask, KMask):
    """Host-side sharding/compaction (see module docstring)."""
    bf16 = ml_dtypes.bfloat16
    qm = np.asarray(QMask).astype(bool)
    km = np.asarray(KMask).astype(bool)
    qidxs = [np.nonzero(qm[b])[0] for b in range(B)]
    kidxs = [np.nonzero(km[b])[0] for b in range(B)]
    maxq = max(len(ix) for ix in qidxs)
    qcap = maxq
    if maxq > 512 and maxq % 512 <= 64:
        qcap = (maxq // 512) * 512
    qidxs = [ix[:qcap] for ix in qidxs]
    ntq = max(1, math.ceil(max(len(ix) for ix in qidxs) / P))
    ntk = max(1, math.ceil(max(len(ix) for ix in kidxs) / P))
    Lq, Lk = ntq * P, ntk * P

    w2 = np.zeros((P, NCH, P), dtype=np.float32)
    hl = np.asarray(HeadLinear, dtype=np.float32)
    for c in range(NCH):
        w2[0:64, c, 0:64] = hl[2 * c]
        w2[64:128, c, 64:128] = hl[2 * c + 1]
    w2b = w2.astype(bf16)
    ob = np.asarray(OutputLiner, dtype=np.float32).astype(bf16)

    in_maps = []
    for b in range(B):
        qi, ki = qidxs[b], kidxs[b]
        qkv = np.zeros((Lq + 2 * Lk, E), dtype=np.float32)
        qkv[:len(qi)] = np.asarray(Q[b], dtype=np.float32)[qi]
        qkv[Lq:Lq + len(ki)] = np.asarray(K[b], dtype=np.float32)[ki]
        qkv[Lq + Lk:Lq + Lk + len(ki)] = np.asarray(V[b], dtype=np.float32)[ki]
        kmc = np.zeros(Lk, dtype=np.float32)
        kmc[:len(ki)] = 1.0
        in_maps.append({
            "QKV": np.ascontiguousarray(qkv.T.astype(bf16)),
            "W2": w2b, "OB": ob,
            "KM": np.ascontiguousarray(kmc.reshape(ntk, P).T),
        })
    return in_maps, qidxs, ntq, ntk


_NC_CACHE = {}


def _get_nc(ntq, ntk):
    if (ntq, ntk) not in _NC_CACHE:
        _NC_CACHE[(ntq, ntk)] = build_bass(ntq, ntk)
    return _NC_CACHE[(ntq, ntk)]


def _host_tail(Q, K, V, HeadLinear, OutputLiner, KMask, b, tidx):
    """Exact fp32 attention for a few overflow queries of batch b."""
    hl = np.asarray(HeadLinear, dtype=np.float32)
    ob = np.asarray(OutputLiner, dtype=np.float32)
    ki = np.nonzero(np.asarray(KMask[b]).astype(bool))[0]
    q = np.asarray(Q[b], dtype=np.float32)[tidx]
    kk = np.asarray(K[b], dtype=np.float32)[ki]
    vv = np.asarray(V[b], dtype=np.float32)[ki]
    outs = []
    for h in range(H):
        sl = slice(h * D, (h + 1) * D)
        qh = q[:, sl] @ hl[h]
        kh = kk[:, sl] @ hl[h]
        vh = vv[:, sl] @ hl[h]
        s = (qh @ kh.T) / np.float32(np.sqrt(D))
        s -= s.max(axis=1, keepdims=True)
        p = np.exp(s)
        p /= p.sum(axis=1, keepdims=True)
        outs.append(p @ vh)
    return np.concatenate(outs, axis=1) @ ob


def kernel(Q, K, V, HeadLinear, OutputLiner, QMask, KMask):
    from concourse.bass_utils import run_bass_kernel_spmd

    in_maps, qidxs, ntq, ntk = make_core_inputs(
        Q, K, V, HeadLinear, OutputLiner, QMask, KMask)
    nc = _get_nc(ntq, ntk)
    res = run_bass_kernel_spmd(nc, in_maps, list(range(B)))
    out = np.zeros((B, L, E), dtype=np.float32)
    qm = np.asarray(QMask).astype(bool)
    bf16 = ml_dtypes.bfloat16
    # last-chunk normalize correction: the device used UNNORMALIZED ct
    # for the final e-chunk's contribution; add (ctN - ctU)^T @ OB_rows
    ob7 = np.asarray(OutputLiner, dtype=np.float32).astype(bf16).astype(
        np.float32)[(NCH - 1) * P:, :]
    for b in range(B):
        yc = np.asarray(res.results[b]["Y"]).astype(np.float32)
        ct7 = np.asarray(res.results[b]["CT7"]).astype(np.float32)
        den = np.asarray(res.results[b]["DEN"]).astype(np.float32)
        scale = np.repeat(1.0 / den, 64, axis=0)            # [128, Lq]
        yc = yc + (ct7 * scale).T @ ob7
        out[b][qidxs[b]] = yc[:len(qidxs[b])]
        full = np.nonzero(qm[b])[0]
        tidx = full[len(qidxs[b]):]
        if len(tidx):
            out[b][tidx] = _host_tail(
                Q, K, V, HeadLinear, OutputLiner, KMask, b, tidx)
    return out


# revision 50
# speedup vs baseline: 1.3008x; 1.0331x over previous
"""Trainium2 Bass kernel for the nn_MultiHeadAttention problem.

Data-parallel over batch: each of the 8 NeuronCores processes one batch
element independently (no collectives).

Mask compaction: the host gathers only the valid query/key positions
(QMask/KMask true), padded to a multiple of 128, and scatters the
output back (masked query rows are exactly zero in the reference).
With ~50% random masks this cuts the attention work ~4x.  If the max
query count only slightly exceeds a 512 multiple, the device is capped
there and the few overflow queries are computed exactly on the host.

V2 engine balance (per core, Lq=512, Lk=640, H=16 heads):
  PE     ~54us: proj + scores + PV + out-proj, software-pipelined so it
         streams continuously (HAM stays warm at 2.4 GHz).
  ACT    ~48us: exp ONLY, merged into 3 ACTIVATEs per head over 2-bank
         [128,1024] PSUM reads to amortize the ~293ns/instr overhead.
  DVE    ~50us: all PSUM evacuation (proj/vproj/pv/den), batched recip,
         batch-1/2 normalize muls, ysum copies/adds.
  GPSIMD: v2 ones-column copies, batch-0 normalize muls, some DMA
         triggers (obs, rbounce/bcs b0, den h<8).
  Sync:  input chunk DMA triggers, den h>=8, Y out.

Normalize: denominator rows DMA-transposed into dstacks, batched
reciprocal on DVE; recip broadcast across partitions via one batched
DRAM-bounce DMA per batch (b0, b1); the final batch (last 2 heads) uses
tiny PE one-hot matmuls into PSUM instead, killing the DMA round-trip
latency on the tail.  Y is written bf16 per q-tile (host casts to f32).
"""

import math
import os
import sys

import numpy as np

try:
    import concourse  # noqa: F401
except ImportError:  # pragma: no cover
    for _p in ("/opt/trn_rl_repo", os.path.expanduser("~/.axon_site/_ro/trn_rl_repo")):
        if os.path.isdir(_p) and _p not in sys.path:
            sys.path.insert(0, _p)

import ml_dtypes

import concourse.bass as bass
import concourse.tile as tile
from concourse import bacc, mybir

B, L, E, H, D = 8, 1024, 1024, 16, 64
P = 128          # partitions
NCH = E // P     # 8 e-chunks (2 heads each)
F32 = mybir.dt.float32
BF16 = mybir.dt.bfloat16


def _chunks(n, step=512):
    return [(s, min(s + step, n)) for s in range(0, n, step)]


def build_bass(ntq, ntk):
    Lq, Lk = ntq * P, ntk * P
    nc = bacc.Bacc(None, target_bir_lowering=False, debug=False)

    Lqkv = Lq + 2 * Lk
    QKV = nc.declare_dram_parameter("QKV", [E, Lqkv], BF16, isOutput=False)
    W2 = nc.declare_dram_parameter("W2", [P, NCH, P], BF16, isOutput=False)
    OB = nc.declare_dram_parameter("OB", [E, E], BF16, isOutput=False)
    KM = nc.declare_dram_parameter("KM", [P, ntk], F32, isOutput=False)
    Y = nc.declare_dram_parameter("Y", [Lq, E], BF16, isOutput=True)
    CT7 = nc.declare_dram_parameter("CT7", [P, Lq], BF16, isOutput=True)
    DEN = nc.declare_dram_parameter("DEN", [2, Lq], F32, isOutput=True)
    rbounce = nc.dram_tensor("rbounce", [H, Lq], BF16)

    # normalize batches: heads [h0,h1) of the first batches are normalized
    # on device once their PVs land (the last one kept small so partA's
    # gate closes early); the final 2 heads stay UNNORMALIZED on device
    # (ct7/dens shipped out, the host applies the exact rank-128
    # correction to Y), killing the tail's normalize latency chain.
    NB = [(0, 8, None), (8, 12, None), (12, 14, None), (H - 2, H, None)]

    with tile.TileContext(nc) as tc:
        with (
            tc.tile_pool(name="singles", bufs=1) as singles,
            tc.tile_pool(name="work", bufs=2) as work,
            tc.tile_pool(name="psum", bufs=2, space="PSUM") as psum,
        ):
            qkT = vaug = ppool = dtpool = work
            ystage = work
            psbig = pspv = pssmall = psum
            # --- persistent SBUF tensors -------------------------------
            qkvts = singles.tile([P, NCH, Lqkv], BF16)
            qts = qkvts[:, :, 0:Lq]
            kts = qkvts[:, :, Lq:Lq + Lk]
            vts = qkvts[:, :, Lq + Lk:Lqkv]
            obs = singles.tile([P, NCH, E], BF16)
            w2s = singles.tile([P, NCH, P], BF16)
            kms = singles.tile([P, ntk], F32)
            ct = singles.tile([P, NCH, Lq], BF16)
            dstacks, rstacks, bstacks = [], [], []
            for bi, (h0, h1, _) in enumerate(NB[:3]):
                nh = h1 - h0
                ds = singles.tile([nh * ntq, P], F32, tag=f"ds{bi}")
                rs = singles.tile([nh * ntq, P], BF16, tag=f"rs{bi}")
                dstacks.append(ds)
                rstacks.append(rs)
                bs = singles.tile([P, nh, Lq], BF16, tag=f"bs{bi}")
                bstacks.append(bs)

            # --- table load + input DMAs -------------------------------
            # dummy exp first so the ~2.7us ACT table load overlaps DMAs
            dume = singles.tile([P, 8], BF16)
            nc.vector.memset(dume[:], 0.0)
            nc.scalar.activation(out=dume[:], in_=dume[:],
                                 func=mybir.ActivationFunctionType.Exp)
            nc.gpsimd.dma_start(out=kms[:], in_=KM[:])
            # PE warmup: cold dummy matmuls so the HAM clock gate opens
            # while the first chunk DMAs land (~3us of cold N=256 MMs)
            warm = singles.tile([P, 256], BF16)
            nc.vector.memset(warm[:], 0.0)
            for wi in range(20):
                wps = pssmall.tile([P, 512], F32, tag="small")
                nc.tensor.matmul(out=wps[:, 0:256], lhsT=warm[:, 0:128],
                                 rhs=warm[:], start=True, stop=True)
            # inputs as small per-chunk pieces in COMPUTE order across two
            # queues (a single queue moves only ~130 GB/s, and chunk 0 must
            # land first): sync carries w2-chunk + q + k, gpsimd carries v.
            for c in range(NCH):
                nc.sync.dma_start(out=w2s[:, c, :],
                                  in_=W2[:, c, :])
                nc.sync.dma_start(out=qkvts[:, c, 0:Lq],
                                  in_=QKV[c * P:(c + 1) * P, 0:Lq])
                nc.sync.dma_start(out=qkvts[:, c, Lq:Lq + Lk],
                                  in_=QKV[c * P:(c + 1) * P, Lq:Lq + Lk])
                nc.gpsimd.dma_start(out=qkvts[:, c, Lq + Lk:Lqkv],
                                    in_=QKV[c * P:(c + 1) * P, Lq + Lk:Lqkv])
            # output-proj weights queue behind the v's (needed only late)
            for c in range(NCH):
                nc.gpsimd.dma_start(out=obs[:, c, :],
                                    in_=OB[c * P:(c + 1) * P, :])

            def batch_of(h):
                return next(i for i, (a, b2, _) in enumerate(NB) if a <= h < b2)

            def emit_pv(h):
                """PV for head h (pt/v2 already computed), plus evacuation."""
                c, hf = h // 2, h % 2
                pv = pspv.tile([65, Lq], F32, tag="pv")
                for kt in range(ntk):
                    nc.tensor.matmul(
                        out=pv[:],
                        lhsT=v2s[c % 2][:, kt, 65 * hf:65 * hf + 65],
                        rhs=pts[c % 2][hf][:, kt, :],
                        start=(kt == 0), stop=(kt == ntk - 1),
                    )
                # evacuate: unnormalized C^T rows + denominator row
                nc.vector.tensor_copy(ct[64 * hf:64 * hf + 64, c, :], pv[0:64, :])
                dtmp = dtpool.tile([1, Lq], F32)
                nc.vector.tensor_copy(dtmp[:], pv[64:65, :])
                bi = batch_of(h)
                hrel = h - NB[bi][0]
                if bi < 3:
                    eng = nc.gpsimd if h < H // 2 else nc.sync
                    eng.dma_start(
                        out=dstacks[bi][hrel * ntq:(hrel + 1) * ntq, :],
                        in_=dtmp[:])
                else:
                    # last 2 heads: denominator goes to the host
                    nc.sync.dma_start(out=DEN[hrel:hrel + 1, :], in_=dtmp[:])

            def emit_norm_batch(bi):
                h0, h1, _ = NB[bi]
                nh = h1 - h0
                with nc.allow_low_precision(reason="softmax recip bf16"):
                    nc.vector.reciprocal(out=rstacks[bi][:], in_=dstacks[bi][:])
                # DRAM bounce + ONE batched broadcast DMA for the batch
                eng = nc.gpsimd if bi == 0 else nc.sync
                eng.dma_start(out=rbounce[h0:h1, :], in_=rstacks[bi][:])
                src = rbounce[h0:h1, :]
                bc_in = bass.AP(
                    tensor=src.tensor, offset=src.offset,
                    ap=[[0, P], [Lq, nh], [1, Lq]])
                eng.dma_start(out=bstacks[bi][:], in_=bc_in)
                for h in range(h0, h1):
                    c, hf = h // 2, h % 2
                    sl = ct[64 * hf:64 * hf + 64, c, :]
                    bsl = bstacks[bi][64 * hf:64 * hf + 64, h - h0, :]
                    # early batches ride the idle gpsimd engine; the last
                    # one gates partA so keep it on the faster DVE
                    meng = nc.gpsimd if bi < 2 else nc.vector
                    meng.tensor_mul(sl, sl, bsl)

            # --- main loop over e-chunks (2 heads each) ----------------
            v2s = [None, None]   # v2 tiles by chunk parity
            pts = [[None, None], [None, None]]  # pt tiles [c%2][hf]
            pending = []
            for c in range(NCH):
                # fused q/k projection for both heads of this chunk
                qkt2 = qkT.tile([P, Lq + Lk], BF16, tag="qkt2")
                for s0, s1 in _chunks(Lq + Lk):
                    ps = pssmall.tile([P, 512], F32, tag="small")
                    if s1 <= Lq:
                        nc.tensor.matmul(
                            out=ps[:, 0:s1 - s0], lhsT=w2s[:, c, :],
                            rhs=qts[:, c, s0:s1], start=True, stop=True)
                    elif s0 >= Lq:
                        nc.tensor.matmul(
                            out=ps[:, 0:s1 - s0], lhsT=w2s[:, c, :],
                            rhs=kts[:, c, s0 - Lq:s1 - Lq], start=True, stop=True)
                    else:
                        mid = Lq - s0
                        nc.tensor.matmul(
                            out=ps[:, 0:mid], lhsT=w2s[:, c, :],
                            rhs=qts[:, c, s0:Lq], start=True, stop=True)
                        nc.tensor.matmul(
                            out=ps[:, mid:s1 - s0], lhsT=w2s[:, c, :],
                            rhs=kts[:, c, 0:s1 - Lq], start=True, stop=True)
                    nc.vector.tensor_copy(qkt2[:, s0:s1], ps[:, 0:s1 - s0])
                qt2 = qkt2[:, 0:Lq]
                kt2 = qkt2[:, Lq:Lq + Lk]

                # v projection: k-tiles in 512-wide PSUM groups, evacuated
                # with strided APs into the [65,2] interleaved v2 layout
                v2 = vaug.tile([P, ntk, 130], BF16)
                v2s[c % 2] = v2
                for t0, t1 in _chunks(ntk * P):
                    ps = pssmall.tile([P, 512], F32, tag="small")
                    for t in range(t0 // P, (t1 + P - 1) // P):
                        nc.tensor.matmul(
                            out=ps[:, t * P - t0:(t + 1) * P - t0],
                            lhsT=vts[:, c, t * P:(t + 1) * P],
                            rhs=w2s[:, c, :],
                            start=True, stop=True,
                        )
                    nt = (t1 - t0) // P
                    base = v2[:, t0 // P, 0:64]
                    vt_out = bass.AP(
                        tensor=base.tensor, offset=base.offset,
                        ap=[list(base.ap[0]), [130, nt], [65, 2], [1, 64]])
                    nc.vector.tensor_copy(
                        vt_out,
                        ps[:, 0:t1 - t0].rearrange(
                            "p (nt two d) -> p nt two d", nt=nt, two=2))
                # denominator "ones" columns = slot-validity mask
                nc.gpsimd.tensor_copy(v2[:, :, 64], kms[:, :])
                nc.gpsimd.tensor_copy(v2[:, :, 129], kms[:, :])

                for hf in range(2):
                    h = 2 * c + hf
                    hq = qt2[64 * hf:64 * hf + 64, :]
                    hk = kt2[64 * hf:64 * hf + 64, :]
                    # scores (transposed, [k, q]) in 2-bank tiles + merged exp
                    pt = ppool.tile([P, ntk, Lq], BF16, tag=f"pt{hf}")
                    pts[c % 2][hf] = pt
                    for t0 in range(0, ntk, 2):
                        t1 = min(t0 + 2, ntk)
                        sps = psbig.tile([P, 1024], F32, tag="big")
                        for t in range(t0, t1):
                            for s0, s1 in _chunks(Lq):
                                nc.tensor.matmul(
                                    out=sps[:, (t - t0) * Lq + s0:(t - t0) * Lq + s1],
                                    lhsT=hk[:, t * P:(t + 1) * P],
                                    rhs=hq[:, s0:s1],
                                    start=True, stop=True,
                                )
                        w = (t1 - t0) * Lq
                        nc.scalar.activation(
                            out=pt[:, t0:t1, :], in_=sps[:, 0:w],
                            func=mybir.ActivationFunctionType.Exp,
                            scale=0.125,
                        )
                    # software pipeline: PV runs one head behind scores
                    pending.append(h)
                    if len(pending) > 1:
                        ph = pending.pop(0)
                        emit_pv(ph)
                        for bi, (_, b1_, _) in enumerate(NB[:3]):
                            if ph == b1_ - 1:
                                emit_norm_batch(bi)

            # output projection over chunks 0..NCH-2 (all device-normalized;
            # the host adds the last chunk's term from CT7/DEN).  partA
            # tile 0 is emitted between the last two PVs so its matmuls
            # fill the PE's wait for the final exps.
            def emit_parta(t):
                ya0 = pssmall.tile([P, 512], F32, tag="small")
                ya1 = pssmall.tile([P, 512], F32, tag="small")
                for c in range(NCH - 1):
                    for eh in range(2):
                        nc.tensor.matmul(
                            out=[ya0, ya1][eh][:],
                            lhsT=ct[:, c, t * P:(t + 1) * P],
                            rhs=obs[:, c, 512 * eh:512 * (eh + 1)],
                            start=(c == 0), stop=(c == NCH - 2),
                        )
                ys = ystage.tile([P, E], BF16, tag="ys", bufs=4)
                if t % 2 == 0:
                    nc.scalar.copy(ys[:, 0:512], ya0[:])
                    nc.vector.tensor_copy(ys[:, 512:1024], ya1[:])
                else:
                    nc.vector.tensor_copy(ys[:, 0:512], ya0[:])
                    nc.scalar.copy(ys[:, 512:1024], ya1[:])
                yeng = nc.sync if t % 2 == 0 else nc.gpsimd
                yeng.dma_start(out=Y[t * P:(t + 1) * P, :], in_=ys[:])

            # partA tile 0 fills the PE while ACT finishes the last exps
            emit_parta(0)
            for ph in pending:
                emit_pv(ph)
                for bi, (_, b1_, _) in enumerate(NB[:3]):
                    if ph == b1_ - 1:
                        emit_norm_batch(bi)
            # ship the unnormalized last chunk to the host for correction
            nc.sync.dma_start(out=CT7[:], in_=ct[:, NCH - 1, :])
            for t in range(1, ntq):
                emit_parta(t)

    nc.compile()
    return nc


def make_core_inputs(Q, K, V, HeadLinear, OutputLiner, QMask, KMask):
    """Host-side sharding/compaction (see module docstring)."""
    bf16 = ml_dtypes.bfloat16
    qm = np.asarray(QMask).astype(bool)
    km = np.asarray(KMask).astype(bool)
    qidxs = [np.nonzero(qm[b])[0] for b in range(B)]
    kidxs = [np.nonzero(km[b])[0] for b in range(B)]
    maxq = max(len(ix) for ix in qidxs)
    qcap = maxq
    if maxq > 512 and maxq % 512 <= 64:
        qcap = (maxq // 512) * 512
    qidxs = [ix[:qcap] for ix in qidxs]
    ntq = max(1, math.ceil(max(len(ix) for ix in qidxs) / P))
    ntk = max(1, math.ceil(max(len(ix) for ix in kidxs) / P))
    Lq, Lk = ntq * P, ntk * P

    w2 = np.zeros((P, NCH, P), dtype=np.float32)
    hl = np.asarray(HeadLinear, dtype=np.float32)
    for c in range(NCH):
        w2[0:64, c, 0:64] = hl[2 * c]
        w2[64:128, c, 64:128] = hl[2 * c + 1]
    w2b = w2.astype(bf16)
    ob = np.asarray(OutputLiner, dtype=np.float32).astype(bf16)

    in_maps = []
    for b in range(B):
        qi, ki = qidxs[b], kidxs[b]
        qkv = np.zeros((Lq + 2 * Lk, E), dtype=np.float32)
        qkv[:len(qi)] = np.asarray(Q[b], dtype=np.float32)[qi]
        qkv[Lq:Lq + len(ki)] = np.asarray(K[b], dtype=np.float32)[ki]
        qkv[Lq + Lk:Lq + Lk + len(ki)] = np.asarray(V[b], dtype=np.float32)[ki]
        kmc = np.zeros(Lk, dtype=np.float32)
        kmc[:len(ki)] = 1.0
        in_maps.append({
            "QKV": np.ascontiguousarray(qkv.T.astype(bf16)),
            "W2": w2b, "OB": ob,
            "KM": np.ascontiguousarray(kmc.reshape(ntk, P).T),
        })
    return in_maps, qidxs, ntq, ntk


_NC_CACHE = {}


def _get_nc(ntq, ntk):
    if (ntq, ntk) not in _NC_CACHE:
        _NC_CACHE[(ntq, ntk)] = build_bass(ntq, ntk)
    return _NC_CACHE[(ntq, ntk)]


def _host_tail(Q, K, V, HeadLinear, OutputLiner, KMask, b, tidx):
    """Exact fp32 attention for a few overflow queries of batch b."""
    hl = np.asarray(HeadLinear, dtype=np.float32)
    ob = np.asarray(OutputLiner, dtype=np.float32)
    ki = np.nonzero(np.asarray(KMask[b]).astype(bool))[0]
    q = np.asarray(Q[b], dtype=np.float32)[tidx]
    kk = np.asarray(K[b], dtype=np.float32)[ki]
    vv = np.asarray(V[b], dtype=np.float32)[ki]
    outs = []
    for h in range(H):
        sl = slice(h * D, (h + 1) * D)
        qh = q[:, sl] @ hl[h]
        kh = kk[:, sl] @ hl[h]
        vh = vv[:, sl] @ hl[h]
        s = (qh @ kh.T) / np.float32(np.sqrt(D))
        s -= s.max(axis=1, keepdims=True)
        p = np.exp(s)
        p /= p.sum(axis=1, keepdims=True)
        outs.append(p @ vh)
    return np.concatenate(outs, axis=1) @ ob


def kernel(Q, K, V, HeadLinear, OutputLiner, QMask, KMask):
    from concourse.bass_utils import run_bass_kernel_spmd

    in_maps, qidxs, ntq, ntk = make_core_inputs(
        Q, K, V, HeadLinear, OutputLiner, QMask, KMask)
    nc = _get_nc(ntq, ntk)
    res = run_bass_kernel_spmd(nc, in_maps, list(range(B)))
    out = np.zeros((B, L, E), dtype=np.float32)
    qm = np.asarray(QMask).astype(bool)
    bf16 = ml_dtypes.bfloat16
    # last-chunk normalize correction: the device used UNNORMALIZED ct
    # for the final e-chunk's contribution; add (ctN - ctU)^T @ OB_rows
    ob7 = np.asarray(OutputLiner, dtype=np.float32).astype(bf16).astype(
        np.float32)[(NCH - 1) * P:, :]
    for b in range(B):
        yc = np.asarray(res.results[b]["Y"]).astype(np.float32)
        ct7 = np.asarray(res.results[b]["CT7"]).astype(np.float32)
        den = np.asarray(res.results[b]["DEN"]).astype(np.float32)
        scale = np.repeat(1.0 / den, 64, axis=0)            # [128, Lq]
        yc = yc + (ct7 * scale).T @ ob7
        out[b][qidxs[b]] = yc[:len(qidxs[b])]
        full = np.nonzero(qm[b])[0]
        tidx = full[len(qidxs[b]):]
        if len(tidx):
            out[b][tidx] = _host_tail(
                Q, K, V, HeadLinear, OutputLiner, KMask, b, tidx)
    return out


# revision 59
# speedup vs baseline: 1.3113x; 1.0081x over previous
"""Trainium2 Bass kernel for the nn_MultiHeadAttention problem.

Data-parallel over batch: each of the 8 NeuronCores processes one batch
element independently (no collectives).

Mask compaction: the host gathers only the valid query/key positions
(QMask/KMask true), padded to a multiple of 128, and scatters the
output back (masked query rows are exactly zero in the reference).
With ~50% random masks this cuts the attention work ~4x.  If the max
query count only slightly exceeds a 512 multiple, the device is capped
there and the few overflow queries are computed exactly on the host.

V2 engine balance (per core, Lq=512, Lk=640, H=16 heads):
  PE     ~54us: proj + scores + PV + out-proj, software-pipelined so it
         streams continuously (HAM stays warm at 2.4 GHz).
  ACT    ~48us: exp ONLY, merged into 3 ACTIVATEs per head over 2-bank
         [128,1024] PSUM reads to amortize the ~293ns/instr overhead.
  DVE    ~50us: all PSUM evacuation (proj/vproj/pv/den), batched recip,
         batch-1/2 normalize muls, ysum copies/adds.
  GPSIMD: v2 ones-column copies, batch-0 normalize muls, some DMA
         triggers (obs, rbounce/bcs b0, den h<8).
  Sync:  input chunk DMA triggers, den h>=8, Y out.

Normalize: denominator rows DMA-transposed into dstacks, batched
reciprocal on DVE; recip broadcast across partitions via one batched
DRAM-bounce DMA per batch (b0, b1); the final batch (last 2 heads) uses
tiny PE one-hot matmuls into PSUM instead, killing the DMA round-trip
latency on the tail.  Y is written bf16 per q-tile (host casts to f32).
"""

import math
import os
import sys

import numpy as np

try:
    import concourse  # noqa: F401
except ImportError:  # pragma: no cover
    for _p in ("/opt/trn_rl_repo", os.path.expanduser("~/.axon_site/_ro/trn_rl_repo")):
        if os.path.isdir(_p) and _p not in sys.path:
            sys.path.insert(0, _p)

import ml_dtypes

import concourse.bass as bass
import concourse.tile as tile
from concourse import bacc, mybir

B, L, E, H, D = 8, 1024, 1024, 16, 64
P = 128          # partitions
NCH = E // P     # 8 e-chunks (2 heads each)
F32 = mybir.dt.float32
BF16 = mybir.dt.bfloat16


def _chunks(n, step=512):
    return [(s, min(s + step, n)) for s in range(0, n, step)]


def build_bass(ntq, ntk):
    Lq, Lk = ntq * P, ntk * P
    nc = bacc.Bacc(None, target_bir_lowering=False, debug=False)

    Lqkv = Lq + 2 * Lk
    QKV = nc.declare_dram_parameter("QKV", [E, Lqkv], BF16, isOutput=False)
    W2 = nc.declare_dram_parameter("W2", [P, NCH, P], BF16, isOutput=False)
    OB = nc.declare_dram_parameter("OB", [E, E], BF16, isOutput=False)
    KM = nc.declare_dram_parameter("KM", [P, ntk], F32, isOutput=False)
    SEL = nc.declare_dram_parameter("SEL", [2 * ntq, 2 * ntq * P], BF16,
                                    isOutput=False)
    Y = nc.declare_dram_parameter("Y", [Lq, E], BF16, isOutput=True)
    CT7 = nc.declare_dram_parameter("CT7", [P, Lq], BF16, isOutput=True)
    DEN = nc.declare_dram_parameter("DEN", [2, Lq], F32, isOutput=True)
    rbounce = nc.dram_tensor("rbounce", [H, Lq], BF16)

    # normalize batches: heads [h0,h1) of the first batches are normalized
    # on device once their PVs land (the last one kept small so partA's
    # gate closes early); the final 2 heads stay UNNORMALIZED on device
    # (ct7/dens shipped out, the host applies the exact rank-128
    # correction to Y), killing the tail's normalize latency chain.
    NB = [(0, 8, None), (8, 12, None), (12, 14, None), (H - 2, H, None)]

    with tile.TileContext(nc) as tc:
        with (
            tc.tile_pool(name="singles", bufs=1) as singles,
            tc.tile_pool(name="work", bufs=2) as work,
            tc.tile_pool(name="psum", bufs=2, space="PSUM") as psum,
        ):
            qkT = vaug = ppool = dtpool = work
            ystage = work
            psbig = pspv = pssmall = psum
            # --- persistent SBUF tensors -------------------------------
            qkvts = singles.tile([P, NCH, Lqkv], BF16)
            qts = qkvts[:, :, 0:Lq]
            kts = qkvts[:, :, Lq:Lq + Lk]
            vts = qkvts[:, :, Lq + Lk:Lqkv]
            obs = singles.tile([P, NCH, E], BF16)
            w2s = singles.tile([P, NCH, P], BF16)
            kms = singles.tile([P, ntk], F32)
            ct = singles.tile([P, NCH, Lq], BF16)
            sel8 = singles.tile([2 * ntq, 2 * ntq * P], BF16)
            dstacks, rstacks, bstacks = [], [], []
            for bi, (h0, h1, _) in enumerate(NB[:3]):
                nh = h1 - h0
                ds = singles.tile([nh * ntq, P], F32, tag=f"ds{bi}")
                rs = singles.tile([nh * ntq, P], BF16, tag=f"rs{bi}")
                dstacks.append(ds)
                rstacks.append(rs)
                bs = singles.tile([P, nh, Lq], BF16, tag=f"bs{bi}")
                bstacks.append(bs)

            # --- table load + input DMAs -------------------------------
            # dummy exp first so the ~2.7us ACT table load overlaps DMAs
            dume = singles.tile([P, 8], BF16)
            nc.vector.memset(dume[:], 0.0)
            nc.scalar.activation(out=dume[:], in_=dume[:],
                                 func=mybir.ActivationFunctionType.Exp)
            nc.gpsimd.dma_start(out=kms[:], in_=KM[:])
            nc.gpsimd.dma_start(out=sel8[:], in_=SEL[:])
            # PE warmup: cold dummy matmuls so the HAM clock gate opens
            # while the first chunk DMAs land (~3us of cold N=256 MMs)
            warm = singles.tile([P, 256], BF16)
            nc.vector.memset(warm[:], 0.0)
            for wi in range(20):
                wps = pssmall.tile([P, 512], F32, tag="small")
                nc.tensor.matmul(out=wps[:, 0:256], lhsT=warm[:, 0:128],
                                 rhs=warm[:], start=True, stop=True)
            # inputs as small per-chunk pieces in COMPUTE order across two
            # queues (a single queue moves only ~130 GB/s, and chunk 0 must
            # land first): sync carries w2-chunk + q + k, gpsimd carries v.
            for c in range(NCH):
                nc.sync.dma_start(out=w2s[:, c, :],
                                  in_=W2[:, c, :])
                nc.sync.dma_start(out=qkvts[:, c, 0:Lq],
                                  in_=QKV[c * P:(c + 1) * P, 0:Lq])
                nc.sync.dma_start(out=qkvts[:, c, Lq:Lq + Lk],
                                  in_=QKV[c * P:(c + 1) * P, Lq:Lq + Lk])
                nc.gpsimd.dma_start(out=qkvts[:, c, Lq + Lk:Lqkv],
                                    in_=QKV[c * P:(c + 1) * P, Lq + Lk:Lqkv])
            # output-proj weights queue behind the v's (needed only late)
            for c in range(NCH):
                nc.gpsimd.dma_start(out=obs[:, c, :],
                                    in_=OB[c * P:(c + 1) * P, :])

            def batch_of(h):
                return next(i for i, (a, b2, _) in enumerate(NB) if a <= h < b2)

            def emit_pv(h):
                """PV for head h (pt/v2 already computed), plus evacuation."""
                c, hf = h // 2, h % 2
                pv = pspv.tile([65, Lq], F32, tag="pv")
                for kt in range(ntk):
                    nc.tensor.matmul(
                        out=pv[:],
                        lhsT=v2s[c % 2][:, kt, 65 * hf:65 * hf + 65],
                        rhs=pts[c % 2][hf][:, kt, :],
                        start=(kt == 0), stop=(kt == ntk - 1),
                    )
                # evacuate: unnormalized C^T rows + denominator row
                nc.vector.tensor_copy(ct[64 * hf:64 * hf + 64, c, :], pv[0:64, :])
                dtmp = dtpool.tile([1, Lq], F32)
                nc.vector.tensor_copy(dtmp[:], pv[64:65, :])
                bi = batch_of(h)
                hrel = h - NB[bi][0]
                if bi < 3:
                    eng = nc.gpsimd if h < H // 2 else nc.sync
                    eng.dma_start(
                        out=dstacks[bi][hrel * ntq:(hrel + 1) * ntq, :],
                        in_=dtmp[:])
                else:
                    # last 2 heads: denominator goes to the host
                    nc.sync.dma_start(out=DEN[hrel:hrel + 1, :], in_=dtmp[:])

            def emit_norm_batch(bi):
                h0, h1, _ = NB[bi]
                nh = h1 - h0
                with nc.allow_low_precision(reason="softmax recip bf16"):
                    nc.vector.reciprocal(out=rstacks[bi][:], in_=dstacks[bi][:])
                if bi < 2:
                    # DRAM bounce + ONE batched broadcast DMA for the batch
                    eng = nc.gpsimd if bi == 0 else nc.sync
                    eng.dma_start(out=rbounce[h0:h1, :], in_=rstacks[bi][:])
                    src = rbounce[h0:h1, :]
                    bc_in = bass.AP(
                        tensor=src.tensor, offset=src.offset,
                        ap=[[0, P], [Lq, nh], [1, Lq]])
                    eng.dma_start(out=bstacks[bi][:], in_=bc_in)
                    for h in range(h0, h1):
                        c, hf = h // 2, h % 2
                        sl = ct[64 * hf:64 * hf + 64, c, :]
                        bsl = bstacks[bi][64 * hf:64 * hf + 64, h - h0, :]
                        # mid-kernel batches ride the idle gpsimd engine
                        nc.gpsimd.tensor_mul(sl, sl, bsl)
                else:
                    # last device batch gates partA: skip the DMA round
                    # trips, broadcast via tiny PE one-hot matmuls instead
                    for h in range(h0, h1):
                        c, hf = h // 2, h % 2
                        hrel = h - h0
                        bc = pssmall.tile([P, Lq], F32, tag="small")
                        for t in range(ntq):
                            r = hrel * ntq + t
                            nc.tensor.matmul(
                                out=bc[:, t * P:(t + 1) * P],
                                lhsT=sel8[:, r * P:(r + 1) * P],
                                rhs=rstacks[bi][:, 0:P],
                                start=True, stop=True)
                        sl = ct[64 * hf:64 * hf + 64, c, :]
                        nc.vector.tensor_mul(sl, sl, bc[64 * hf:64 * hf + 64, :])

            def emit_parta(t):
                ya0 = pssmall.tile([P, 512], F32, tag="small")
                ya1 = pssmall.tile([P, 512], F32, tag="small")
                for c in range(NCH - 1):
                    for eh in range(2):
                        nc.tensor.matmul(
                            out=[ya0, ya1][eh][:],
                            lhsT=ct[:, c, t * P:(t + 1) * P],
                            rhs=obs[:, c, 512 * eh:512 * (eh + 1)],
                            start=(c == 0), stop=(c == NCH - 2),
                        )
                ys = ystage.tile([P, E], BF16, tag="ys", bufs=4)
                if t % 2 == 0:
                    nc.scalar.copy(ys[:, 0:512], ya0[:])
                    nc.vector.tensor_copy(ys[:, 512:1024], ya1[:])
                else:
                    nc.vector.tensor_copy(ys[:, 0:512], ya0[:])
                    nc.scalar.copy(ys[:, 512:1024], ya1[:])
                yeng = nc.sync if t % 2 == 0 else nc.gpsimd
                yeng.dma_start(out=Y[t * P:(t + 1) * P, :], in_=ys[:])


            # --- main loop over e-chunks (2 heads each) ----------------
            v2s = [None, None]   # v2 tiles by chunk parity
            pts = [[None, None], [None, None]]  # pt tiles [c%2][hf]
            pending = []
            for c in range(NCH):
                # fused q/k projection for both heads of this chunk
                qkt2 = qkT.tile([P, Lq + Lk], BF16, tag="qkt2")
                for s0, s1 in _chunks(Lq + Lk):
                    ps = pssmall.tile([P, 512], F32, tag="small")
                    if s1 <= Lq:
                        nc.tensor.matmul(
                            out=ps[:, 0:s1 - s0], lhsT=w2s[:, c, :],
                            rhs=qts[:, c, s0:s1], start=True, stop=True)
                    elif s0 >= Lq:
                        nc.tensor.matmul(
                            out=ps[:, 0:s1 - s0], lhsT=w2s[:, c, :],
                            rhs=kts[:, c, s0 - Lq:s1 - Lq], start=True, stop=True)
                    else:
                        mid = Lq - s0
                        nc.tensor.matmul(
                            out=ps[:, 0:mid], lhsT=w2s[:, c, :],
                            rhs=qts[:, c, s0:Lq], start=True, stop=True)
                        nc.tensor.matmul(
                            out=ps[:, mid:s1 - s0], lhsT=w2s[:, c, :],
                            rhs=kts[:, c, 0:s1 - Lq], start=True, stop=True)
                    nc.vector.tensor_copy(qkt2[:, s0:s1], ps[:, 0:s1 - s0])
                qt2 = qkt2[:, 0:Lq]
                kt2 = qkt2[:, Lq:Lq + Lk]

                # v projection: k-tiles in 512-wide PSUM groups, evacuated
                # with strided APs into the [65,2] interleaved v2 layout
                v2 = vaug.tile([P, ntk, 130], BF16)
                v2s[c % 2] = v2
                for t0, t1 in _chunks(ntk * P):
                    ps = pssmall.tile([P, 512], F32, tag="small")
                    for t in range(t0 // P, (t1 + P - 1) // P):
                        nc.tensor.matmul(
                            out=ps[:, t * P - t0:(t + 1) * P - t0],
                            lhsT=vts[:, c, t * P:(t + 1) * P],
                            rhs=w2s[:, c, :],
                            start=True, stop=True,
                        )
                    nt = (t1 - t0) // P
                    base = v2[:, t0 // P, 0:64]
                    vt_out = bass.AP(
                        tensor=base.tensor, offset=base.offset,
                        ap=[list(base.ap[0]), [130, nt], [65, 2], [1, 64]])
                    nc.vector.tensor_copy(
                        vt_out,
                        ps[:, 0:t1 - t0].rearrange(
                            "p (nt two d) -> p nt two d", nt=nt, two=2))
                # denominator "ones" columns = slot-validity mask
                nc.gpsimd.tensor_copy(v2[:, :, 64], kms[:, :])
                nc.gpsimd.tensor_copy(v2[:, :, 129], kms[:, :])

                for hf in range(2):
                    h = 2 * c + hf
                    hq = qt2[64 * hf:64 * hf + 64, :]
                    hk = kt2[64 * hf:64 * hf + 64, :]
                    # scores (transposed, [k, q]) in 2-bank tiles + merged exp
                    pt = ppool.tile([P, ntk, Lq], BF16, tag=f"pt{hf}")
                    pts[c % 2][hf] = pt
                    for t0 in range(0, ntk, 2):
                        t1 = min(t0 + 2, ntk)
                        sps = psbig.tile([P, 1024], F32, tag="big")
                        for t in range(t0, t1):
                            for s0, s1 in _chunks(Lq):
                                nc.tensor.matmul(
                                    out=sps[:, (t - t0) * Lq + s0:(t - t0) * Lq + s1],
                                    lhsT=hk[:, t * P:(t + 1) * P],
                                    rhs=hq[:, s0:s1],
                                    start=True, stop=True,
                                )
                        w = (t1 - t0) * Lq
                        nc.scalar.activation(
                            out=pt[:, t0:t1, :], in_=sps[:, 0:w],
                            func=mybir.ActivationFunctionType.Exp,
                            scale=0.125,
                        )
                    # software pipeline: PV runs one head behind scores
                    pending.append(h)
                    if len(pending) > 1:
                        ph = pending.pop(0)
                        if ph == H - 2:
                            # fill the PE's wait for the last exps
                            emit_parta(0)
                        emit_pv(ph)
                        for bi, (_, b1_, _) in enumerate(NB[:3]):
                            if ph == b1_ - 1:
                                emit_norm_batch(bi)

            # output projection over chunks 0..NCH-2 (all device-normalized;
            # the host adds the last chunk's term from CT7/DEN).  partA
            # tile 0 is emitted between the last two PVs so its matmuls
            # fill the PE's wait for the final exps.
            emit_parta(1)
            for ph in pending:
                emit_pv(ph)
            # ship the unnormalized last chunk to the host for correction
            nc.sync.dma_start(out=CT7[:], in_=ct[:, NCH - 1, :])
            for t in range(2, ntq):
                emit_parta(t)

    nc.compile()
    return nc


def make_core_inputs(Q, K, V, HeadLinear, OutputLiner, QMask, KMask):
    """Host-side sharding/compaction (see module docstring)."""
    bf16 = ml_dtypes.bfloat16
    qm = np.asarray(QMask).astype(bool)
    km = np.asarray(KMask).astype(bool)
    qidxs = [np.nonzero(qm[b])[0] for b in range(B)]
    kidxs = [np.nonzero(km[b])[0] for b in range(B)]
    maxq = max(len(ix) for ix in qidxs)
    qcap = maxq
    if maxq > 512 and maxq % 512 <= 64:
        qcap = (maxq // 512) * 512
    qidxs = [ix[:qcap] for ix in qidxs]
    ntq = max(1, math.ceil(max(len(ix) for ix in qidxs) / P))
    ntk = max(1, math.ceil(max(len(ix) for ix in kidxs) / P))
    Lq, Lk = ntq * P, ntk * P

    w2 = np.zeros((P, NCH, P), dtype=np.float32)
    hl = np.asarray(HeadLinear, dtype=np.float32)
    for c in range(NCH):
        w2[0:64, c, 0:64] = hl[2 * c]
        w2[64:128, c, 64:128] = hl[2 * c + 1]
    w2b = w2.astype(bf16)
    ob = np.asarray(OutputLiner, dtype=np.float32).astype(bf16)
    nb2 = 2 * ntq
    sel8 = np.zeros((nb2, nb2 * P), dtype=np.float32)
    for r in range(nb2):
        sel8[r, r * P:(r + 1) * P] = 1.0
    sel8 = sel8.astype(bf16)

    in_maps = []
    for b in range(B):
        qi, ki = qidxs[b], kidxs[b]
        qkv = np.zeros((Lq + 2 * Lk, E), dtype=np.float32)
        qkv[:len(qi)] = np.asarray(Q[b], dtype=np.float32)[qi]
        qkv[Lq:Lq + len(ki)] = np.asarray(K[b], dtype=np.float32)[ki]
        qkv[Lq + Lk:Lq + Lk + len(ki)] = np.asarray(V[b], dtype=np.float32)[ki]
        kmc = np.zeros(Lk, dtype=np.float32)
        kmc[:len(ki)] = 1.0
        in_maps.append({
            "QKV": np.ascontiguousarray(qkv.T.astype(bf16)),
            "W2": w2b, "OB": ob,
            "KM": np.ascontiguousarray(kmc.reshape(ntk, P).T),
            "SEL": sel8,
        })
    return in_maps, qidxs, ntq, ntk


_NC_CACHE = {}


def _get_nc(ntq, ntk):
    if (ntq, ntk) not in _NC_CACHE:
        _NC_CACHE[(ntq, ntk)] = build_bass(ntq, ntk)
    return _NC_CACHE[(ntq, ntk)]


def _host_tail(Q, K, V, HeadLinear, OutputLiner, KMask, b, tidx):
    """Exact fp32 attention for a few overflow queries of batch b."""
    hl = np.asarray(HeadLinear, dtype=np.float32)
    ob = np.asarray(OutputLiner, dtype=np.float32)
    ki = np.nonzero(np.asarray(KMask[b]).astype(bool))[0]
    q = np.asarray(Q[b], dtype=np.float32)[tidx]
    kk = np.asarray(K[b], dtype=np.float32)[ki]
    vv = np.asarray(V[b], dtype=np.float32)[ki]
    outs = []
    for h in range(H):
        sl = slice(h * D, (h + 1) * D)
        qh = q[:, sl] @ hl[h]
        kh = kk[:, sl] @ hl[h]
        vh = vv[:, sl] @ hl[h]
        s = (qh @ kh.T) / np.float32(np.sqrt(D))
        s -= s.max(axis=1, keepdims=True)
        p = np.exp(s)
        p /= p.sum(axis=1, keepdims=True)
        outs.append(p @ vh)
    return np.concatenate(outs, axis=1) @ ob


def kernel(Q, K, V, HeadLinear, OutputLiner, QMask, KMask):
    from concourse.bass_utils import run_bass_kernel_spmd

    in_maps, qidxs, ntq, ntk = make_core_inputs(
        Q, K, V, HeadLinear, OutputLiner, QMask, KMask)
    nc = _get_nc(ntq, ntk)
    res = run_bass_kernel_spmd(nc, in_maps, list(range(B)))
    out = np.zeros((B, L, E), dtype=np.float32)
    qm = np.asarray(QMask).astype(bool)
    bf16 = ml_dtypes.bfloat16
    # last-chunk normalize correction: the device used UNNORMALIZED ct
    # for the final e-chunk's contribution; add (ctN - ctU)^T @ OB_rows
    ob7 = np.asarray(OutputLiner, dtype=np.float32).astype(bf16).astype(
        np.float32)[(NCH - 1) * P:, :]
    for b in range(B):
        yc = np.asarray(res.results[b]["Y"]).astype(np.float32)
        ct7 = np.asarray(res.results[b]["CT7"]).astype(np.float32)
        den = np.asarray(res.results[b]["DEN"]).astype(np.float32)
        scale = np.repeat(1.0 / den, 64, axis=0)            # [128, Lq]
        yc = yc + (ct7 * scale).T @ ob7
        out[b][qidxs[b]] = yc[:len(qidxs[b])]
        full = np.nonzero(qm[b])[0]
        tidx = full[len(qidxs[b]):]
        if len(tidx):
            out[b][tidx] = _host_tail(
                Q, K, V, HeadLinear, OutputLiner, KMask, b, tidx)
    return out
